# revision 1
# baseline (speedup 1.0000x reference)
"""Batched CRF Viterbi decode (N=64, C=8, L=32768) on 8 TRN2 NeuronCores.

Self-contained kernel: takes FULL unsharded inputs, shards the batch dim
across 8 cores (data-parallel), runs a Bass/Tile kernel per core, and
gathers the full [64, 32768] int32 path.

Algorithm:
  Phase 1 (parallel two-pass quantized-replay scan): the L-step viterbi
    forward recurrence is split into 128*G independent chunks (128
    partitions x G chunks in the free dim), each warmed up with an
    H-step halo (viterbi relative state coalesces within a few steps).
    Because the reference's fp32 forward variables are large (~6e4),
    its arithmetic is exactly fixed-point on the power-of-2 grid
    ulp(fv); that integer max-plus dynamics is shift-invariant for any
    offset that is a multiple of 2*ulp. Pass 1 runs chunks from zero
    and reconstructs each chunk's absolute frame offset (halo-overlap
    deltas + prefix sum, snapped to a coarse 2^-7 grid). Pass 2 re-runs
    chunks seeded at that absolute magnitude, which replays the
    reference's rounding bit-exactly (validated: 0/2M mismatches).
    The first chunk of each sequence uses an identity-matrix halo so it
    is exact from t=0. Each chunk's entry fv ("seed") is stored so
    phase 2 can recompute scores bit-consistently at chunk boundaries.
  Phase 2 (parallel): backpointers+1 via first-index-of-max recovered
    from the vit series; end-nodes; the length-1 reset folded in.
  Phase 3 (parallel): chunked backward traversal (integer-exact):
    per-chunk candidate trajectories for all 8 entry states,
    hierarchical map composition for chunk entries, final select +
    length mask.
"""
import sys
import numpy as np

if '/opt/trn_rl_repo' not in sys.path:
    sys.path.insert(0, '/opt/trn_rl_repo')

N_FULL, C, L = 64, 8, 32768
SEQ = 8          # sequences per core
NSTRIP = 16      # time strips per core (partition dim = NSTRIP*SEQ = 128)
S = 16           # phase-3 chunk length
NCORES = 8

# phase-1 speculative scan params
SC = 256         # forward chunk length (must be multiple of phase-2 TB)
HALO = 16        # warmup steps per chunk
TB1 = 64         # vit store block (timesteps per DMA)
G_DVE = 8        # chunks per lane handled by the vector engine (rest: gpsimd)
KL_DVE = 128     # phase-3 chunk columns handled by the vector engine
P2_POOL = set()               # phase-2 rounds handled by gpsimd (none: pool lacks max)

_CACHE = {}


def _shapes(L):
    STRIP = L // NSTRIP
    TB = min(128, STRIP)
    return dict(STRIP=STRIP, TB=TB, ROUNDS=STRIP // TB, KL=STRIP // S,
                G=STRIP // SC)


def _host_prep(observes_core, transitions, lengths_core, L):
    sh = _shapes(L)
    STRIP, KL, G = sh["STRIP"], sh["KL"], sh["G"]
    obs_t = np.ascontiguousarray(
        np.transpose(np.asarray(observes_core, np.float32), (0, 2, 1)))
    obs_pad = np.concatenate([np.zeros((SEQ, 1, C), np.float32), obs_t], 1)
    T = np.asarray(transitions, np.float32)
    lens = np.asarray(lengths_core).astype(np.float32)
    p = np.arange(128)

    # phase-1 lane obs: lane p=(s,n), chunk g, step j in [0, SC+HALO):
    #   value = obs_t[n, base+j, :] with base = s*STRIP + g*SC - HALO
    #   (zero when base+j < 0; only chunk (s=0,g=0))
    s_idx = p // SEQ
    n_idx = p % SEQ
    j = np.arange(SC + HALO)
    g = np.arange(G)
    tpos = (s_idx[:, None, None] * STRIP + g[None, :, None] * SC
            - HALO + j[None, None, :])          # [128, G, SC+HALO]
    valid = tpos >= 0
    tcl = np.clip(tpos, 0, L - 1)
    obs_lane = obs_t[n_idx[:, None, None], tcl, :]      # [128, G, SC+HALO, C]
    obs_lane = obs_lane * valid[..., None].astype(np.float32)

    # halo transition matrices: identity-ish for the exact first chunk
    trep_h = np.tile(T.reshape(1, 1, C, C), (128, G, 1, 1)).astype(np.float32)
    ident = np.full((C, C), -1e30, np.float32)
    np.fill_diagonal(ident, 0.0)
    trep_h[:SEQ, 0] = ident                     # chunks (s=0, g=0), all seqs

    return {
        "obs": obs_pad.reshape(SEQ, (L + 1) * C),
        "obs_lane": np.ascontiguousarray(obs_lane).reshape(128, G * (SC + HALO) * C),
        "trep_h": np.ascontiguousarray(trep_h).reshape(128, G * C * C),
        "trep": np.tile(T.reshape(1, C * C), (128, 1)).astype(np.float32),
        "wdesc": np.tile((C - np.arange(C, dtype=np.float32)).reshape(1, C), (128, 1)),
        "tplane": ((p[:, None] // SEQ) * STRIP
                   + np.arange(STRIP)[None, :]).astype(np.float32),
        "len_col": lens[p % SEQ][:, None].astype(np.float32),
        "lenm1": (lens[p % SEQ][:, None] - 1.0).astype(np.float32),
        "einit1": np.tile((np.arange(C, dtype=np.float32)[:, None] + 1.0),
                          (1, KL)).reshape(1, C * KL).repeat(128, 0).astype(np.float32),
    }


def _host_post(path_dev, L):
    STRIP = L // NSTRIP
    return path_dev.reshape(NSTRIP, SEQ, STRIP).transpose(1, 0, 2).reshape(SEQ, L)


def _emit(tc, ins, outs, L):
    import concourse.bass as bass
    import concourse.mybir as mybir
    import bass_rust

    F32 = mybir.dt.float32
    I32 = mybir.dt.int32
    ALU = mybir.AluOpType
    AX = mybir.AxisListType

    def v(ap, off, dims):
        return bass_rust.AP(tensor=ap.tensor, offset=ap.offset + off, ap=dims)

    nc = tc.nc
    sh = _shapes(L)
    STRIP, TB, ROUNDS, KL, G = (sh["STRIP"], sh["TB"], sh["ROUNDS"],
                                sh["KL"], sh["G"])
    G1 = min(8, KL)
    NG = KL // G1
    FLATN = (L + 1) * C
    CH = SC + HALO   # steps per chunk

    obs_d = ins["obs"]
    obs_lane_d = ins["obs_lane"]
    trep_h_d = ins["trep_h"]
    trep_d = ins["trep"]
    wdesc_d = ins["wdesc"]
    tplane_d = ins["tplane"]
    len_d = ins["len_col"]
    lenm1_d = ins["lenm1"]
    einit1_d = ins["einit1"]
    path_d = outs["path"]

    bp1_d = nc.dram_tensor("bp1_scratch", [128, STRIP * C], F32).ap()
    smap_d = nc.dram_tensor("smap_scratch", [128, C], F32).ap()
    estrip_d = nc.dram_tensor("estrip_scratch", [SEQ, NSTRIP], F32).ap()
    s0_d = nc.dram_tensor("s0_scratch", [128, G], F32).ap()
    e0_d = nc.dram_tensor("e0_scratch", [128, G], F32).ap()
    r_d = nc.dram_tensor("r_scratch", [SEQ, NSTRIP * G], F32).ap()

    vec = nc.vector

    with tc.tile_pool(name="const", bufs=1) as cpool:
        trep = cpool.tile([128, C * C], F32)
        wdesc = cpool.tile([128, C], F32)
        tplane = cpool.tile([128, STRIP], F32)
        len_sb = cpool.tile([128, 1], F32)
        lenm1_sb = cpool.tile([128, 1], F32)
        seeds = cpool.tile([128, G * C], F32)
        nc.sync.dma_start(out=trep[:], in_=trep_d)
        nc.sync.dma_start(out=wdesc[:], in_=wdesc_d)
        nc.sync.dma_start(out=tplane[:], in_=tplane_d)
        nc.sync.dma_start(out=len_sb[:], in_=len_d)
        nc.sync.dma_start(out=lenm1_sb[:], in_=lenm1_d)

        # ============ phase 1: two-pass quantized-replay forward scan ============
        K_ALL = NSTRIP * G     # chunks per sequence
        pool_e = nc.gpsimd
        SPLITS = [(vec, 0, G_DVE, "d")]
        if G_DVE < G:
            SPLITS.append((pool_e, G_DVE, G, "p"))
        vitpool_cm = tc.tile_pool(name="vitp", bufs=1)
        vitpool = vitpool_cm.__enter__()
        vit_sb = vitpool.tile([128, STRIP * C], F32)
        with tc.tile_pool(name="ph1c", bufs=1) as ppool:
            obs_lane = ppool.tile([128, G * CH * C], F32)
            trep_h = ppool.tile([128, G * C * C], F32)
            nc.sync.dma_start(out=obs_lane[:], in_=obs_lane_d)
            nc.sync.dma_start(out=trep_h[:], in_=trep_h_d)

            P = lambda t: t[:].ap[0]
            s0 = ppool.tile([128, G], F32)
            e0 = ppool.tile([128, G], F32)

            # per-engine chain state
            st = {}
            for eng, g0, g1, nm in SPLITS:
                ge = g1 - g0
                fv = ppool.tile([128, ge * C], F32, tag="fv" + nm)
                sce = ppool.tile([128, ge * C * C], F32, tag="sc" + nm)
                vtmp = ppool.tile([128, ge * C], F32, tag="vt" + nm)
                st[nm] = dict(
                    eng=eng, g0=g0, g1=g1, ge=ge, fv=fv, sc=sce, vtmp=vtmp,
                    fvb=v(fv[:], 0, [P(fv), [C, ge], [0, C], [1, C]]),
                    treph3=v(trep_h[:], g0 * C * C,
                             [P(trep_h), [C * C, ge], [C, C], [1, C]]),
                    trep3=v(trep[:], 0, [P(trep), [0, ge], [C, C], [1, C]]),
                    sc3=v(sce[:], 0, [P(sce), [C * C, ge], [C, C], [1, C]]),
                    vtmp2=v(vtmp[:], 0, [P(vtmp), [C, ge], [1, C]]),
                )

            def chain(store):
                """Emit one chunked scan pass on both engines. store=False:
                probes only (pass 1). store=True: vit into vit_sb + seeds."""
                for j in range(CH):
                    halo = j < HALO
                    for _, _, _, nm in SPLITS:
                        e = st[nm]
                        eng, ge = e["eng"], e["ge"]
                        eng.tensor_tensor(
                            out=e["sc3"], in0=e["fvb"],
                            in1=(e["treph3"] if halo else e["trep3"]), op=ALU.add)
                        if halo or not store:
                            vcol = e["vtmp2"]
                        else:
                            jr = j - HALO
                            vcol = v(vit_sb[:], (e["g0"] * SC + jr) * C,
                                     [P(vit_sb), [SC * C, ge], [1, C]])
                        eng.tensor_reduce(out=vcol, in_=e["sc3"], axis=AX.X,
                                          op=ALU.max)
                        eng.tensor_tensor(
                            out=e["fv"][:], in0=vcol,
                            in1=v(obs_lane[:], (e["g0"] * CH + j) * C,
                                  [P(obs_lane), [CH * C, ge], [1, C]]),
                            op=ALU.add)
                        if j == HALO - 1:
                            if store:
                                eng.tensor_copy(
                                    out=seeds[:, e["g0"] * C:e["g1"] * C],
                                    in_=e["fv"][:])
                            else:
                                eng.tensor_copy(
                                    out=s0[:, e["g0"]:e["g1"]],
                                    in_=v(e["fv"][:], 0, [P(e["fv"]), [C, ge]]))

            # ---- pass 1: clean chunks from zero; probe frame offsets ----
            for _, _, _, nm in SPLITS:
                st[nm]["eng"].memset(st[nm]["fv"][:], 0.0)
            chain(store=False)
            for _, _, _, nm in SPLITS:
                e = st[nm]
                e["eng"].tensor_copy(out=e0[:, e["g0"]:e["g1"]],
                                     in_=v(e["fv"][:], 0, [P(e["fv"]), [C, e["ge"]]]))
            nc.sync.dma_start(out=e0_d, in_=e0[:])
            nc.sync.dma_start(out=s0_d, in_=s0[:])
            tc.strict_bb_all_engine_barrier()

            # ---- frame offsets: delta -> serial prefix -> snap ----
            # s0_d flat = (s*SEQ+n)*G + g; per-seq view [n, k=s*G+g]
            seq_dims = [[G, SEQ], [SEQ * G, NSTRIP], [1, G]]
            s0_t = ppool.tile([SEQ, K_ALL], F32)
            e0_t = ppool.tile([SEQ, K_ALL], F32)
            nc.sync.dma_start(out=s0_t[:], in_=v(s0_d, 0, seq_dims))
            nc.sync.dma_start(out=e0_t[:], in_=v(e0_d, 0, seq_dims))
            delta = ppool.tile([SEQ, K_ALL], F32)
            vec.memset(delta[:], 0.0)
            vec.tensor_tensor(
                out=delta[:, 0:K_ALL - 1], in0=e0_t[:, 0:K_ALL - 1],
                in1=v(s0_t[:], 1, [P(s0_t), [1, K_ALL - 1]]), op=ALU.subtract)
            r_t = ppool.tile([SEQ, K_ALL], F32)
            vec.memset(r_t[:, 0:1], 0.0)
            for k in range(1, K_ALL):
                vec.tensor_tensor(out=r_t[:, k:k + 1], in0=r_t[:, k - 1:k],
                                  in1=delta[:, k - 1:k], op=ALU.add)
            # snap to the coarse power-of-2 grid (2*ulp at max magnitude)
            vec.tensor_scalar(out=r_t[:], in0=r_t[:], scalar1=98304.0,
                              scalar2=None, op0=ALU.add)
            vec.tensor_scalar(out=r_t[:], in0=r_t[:], scalar1=-98304.0,
                              scalar2=None, op0=ALU.add)
            nc.sync.dma_start(out=r_d, in_=r_t[:])
            tc.strict_bb_all_engine_barrier()

            # ---- pass 2: replay at absolute magnitude ----
            r_sb = ppool.tile([128, G], F32)
            nc.sync.dma_start(
                out=r_sb[:],
                in_=v(r_d, 0, [[G, NSTRIP], [K_ALL, SEQ], [1, G]]))
            for _, _, _, nm in SPLITS:
                e = st[nm]
                e["eng"].tensor_scalar(
                    out=e["fv"][:],
                    in0=v(r_sb[:], e["g0"], [P(r_sb), [1, e["ge"]], [0, C]]),
                    scalar1=0.0, scalar2=None, op0=ALU.add)
            chain(store=True)

        # ============ phase 2: backpointer extraction ============
        with tc.tile_pool(name="ph2", bufs=2) as pool:
            P0 = lambda t: t[:].ap[0]
            for r in range(ROUNDS):
                eng = nc.gpsimd if r in P2_POOL else vec
                off = r * TB * C
                vbase = (r * TB - 1) * C    # vit_sb col for fv window col 0
                obs_blk = pool.tile([128, (TB + 1) * C], F32, tag="obs")
                fv_blk = pool.tile([128, (TB + 1) * C], F32, tag="fv")
                src_dims = [[STRIP * C, NSTRIP], [FLATN, SEQ], [1, (TB + 1) * C]]
                nc.sync.dma_start(out=obs_blk[:], in_=v(obs_d, off, src_dims))
                if r == 0:
                    # col 0 is seed-replaced; vit_sb has no slot for t=0
                    nc.gpsimd.tensor_tensor(
                        out=fv_blk[:, C:(TB + 1) * C],
                        in0=v(vit_sb[:], 0, [P0(vit_sb), [1, TB * C]]),
                        in1=obs_blk[:, C:(TB + 1) * C], op=ALU.add)
                else:
                    nc.gpsimd.tensor_tensor(
                        out=fv_blk[:],
                        in0=v(vit_sb[:], vbase, [P0(vit_sb), [1, (TB + 1) * C]]),
                        in1=obs_blk[:], op=ALU.add)
                if (r * TB) % SC == 0:
                    gi = (r * TB) // SC
                    eng.tensor_copy(out=fv_blk[:, 0:C],
                                    in_=seeds[:, gi * C:(gi + 1) * C])

                P = lambda t: t[:].ap[0]
                sc2 = pool.tile([128, C * TB * C], F32, tag="sc")
                eq2 = sc2
                nc.gpsimd.tensor_tensor(
                    out=sc2[:],
                    in0=v(fv_blk[:], 0, [P(fv_blk), [0, C], [C, TB], [1, C]]),
                    in1=v(trep[:], 0, [P(trep), [C, C], [0, TB], [1, C]]),
                    op=ALU.add)
                eng.tensor_tensor(
                    out=v(eq2[:], 0, [P(eq2), [TB * C, C], [C, TB], [1, C]]),
                    in0=v(sc2[:], 0, [P(sc2), [TB * C, C], [C, TB], [1, C]]),
                    in1=v(vit_sb[:], vbase + C,
                          [P0(vit_sb), [1, C], [C, TB], [0, C]]),
                    op=ALU.is_equal)
                eng.tensor_tensor(
                    out=v(eq2[:], 0, [P(eq2), [TB * C, C], [C, TB], [1, C]]),
                    in0=v(eq2[:], 0, [P(eq2), [TB * C, C], [C, TB], [1, C]]),
                    in1=v(wdesc[:], 0, [P(wdesc), [0, C], [0, TB], [1, C]]),
                    op=ALU.mult)
                bpw = pool.tile([128, C * TB], F32, tag="bpw")
                if eng is vec:
                    eng.tensor_reduce(
                        out=bpw[:],
                        in_=v(eq2[:], 0, [P(eq2), [TB * C, C], [C, TB], [1, C]]),
                        axis=AX.X, op=ALU.max)
                else:
                    # gpsimd has no free-axis reduce: log-tree of pairwise max
                    # (exact: max is associative); scratch reuses dead sc2
                    eng.tensor_tensor(
                        out=v(sc2[:], 0, [P(sc2), [TB * 4, C], [4, TB], [1, 4]]),
                        in0=v(eq2[:], 0, [P(eq2), [TB * C, C], [C, TB], [2, 4]]),
                        in1=v(eq2[:], 1, [P(eq2), [TB * C, C], [C, TB], [2, 4]]),
                        op=ALU.max)
                    eng.tensor_tensor(
                        out=v(sc2[:], C * TB * 4,
                              [P(sc2), [TB * 2, C], [2, TB], [1, 2]]),
                        in0=v(sc2[:], 0, [P(sc2), [TB * 4, C], [4, TB], [2, 2]]),
                        in1=v(sc2[:], 1, [P(sc2), [TB * 4, C], [4, TB], [2, 2]]),
                        op=ALU.max)
                    eng.tensor_tensor(
                        out=v(bpw[:], 0, [P(bpw), [TB, C], [1, TB]]),
                        in0=v(sc2[:], C * TB * 4, [P(sc2), [TB * 2, C], [2, TB]]),
                        in1=v(sc2[:], C * TB * 4 + 1,
                              [P(sc2), [TB * 2, C], [2, TB]]),
                        op=ALU.max)
                bp1 = pool.tile([128, C * TB], F32, tag="bp1")
                eng.tensor_scalar(out=bp1[:], in0=bpw[:], scalar1=-1.0, scalar2=9.0,
                                  op0=ALU.mult, op1=ALU.add)

                fm = pool.tile([128, TB], F32, tag="fm")
                f1 = pool.tile([128, TB * 4], F32, tag="f1", bufs=1)
                f2 = pool.tile([128, TB * 2], F32, tag="f2", bufs=1)
                if eng is vec:
                    eng.tensor_reduce(
                        out=fm[:],
                        in_=v(fv_blk[:], C, [P(fv_blk), [C, TB], [1, C]]),
                        axis=AX.X, op=ALU.max)
                else:
                    eng.tensor_tensor(
                        out=v(f1[:], 0, [P(f1), [4, TB], [1, 4]]),
                        in0=v(fv_blk[:], C, [P(fv_blk), [C, TB], [2, 4]]),
                        in1=v(fv_blk[:], C + 1, [P(fv_blk), [C, TB], [2, 4]]),
                        op=ALU.max)
                    eng.tensor_tensor(
                        out=v(f2[:], 0, [P(f2), [2, TB], [1, 2]]),
                        in0=v(f1[:], 0, [P(f1), [4, TB], [2, 2]]),
                        in1=v(f1[:], 1, [P(f1), [4, TB], [2, 2]]),
                        op=ALU.max)
                    eng.tensor_tensor(
                        out=fm[:],
                        in0=v(f2[:], 0, [P(f2), [2, TB]]),
                        in1=v(f2[:], 1, [P(f2), [2, TB]]),
                        op=ALU.max)
                eqn = pool.tile([128, TB * C], F32, tag="eqn")
                eng.tensor_tensor(
                    out=eqn[:],
                    in0=v(fv_blk[:], C, [P(fv_blk), [C, TB], [1, C]]),
                    in1=v(fm[:], 0, [P(fm), [1, TB], [0, C]]),
                    op=ALU.is_equal)
                eng.tensor_tensor(
                    out=eqn[:],
                    in0=v(eqn[:], 0, [P(eqn), [C, TB], [1, C]]),
                    in1=v(wdesc[:], 0, [P(wdesc), [0, TB], [1, C]]),
                    op=ALU.mult)
                mn = pool.tile([128, TB], F32, tag="mn")
                if eng is vec:
                    eng.tensor_reduce(
                        out=mn[:],
                        in_=v(eqn[:], 0, [P(eqn), [C, TB], [1, C]]),
                        axis=AX.X, op=ALU.max)
                else:
                    eng.tensor_tensor(
                        out=v(f1[:], 0, [P(f1), [4, TB], [1, 4]]),
                        in0=v(eqn[:], 0, [P(eqn), [C, TB], [2, 4]]),
                        in1=v(eqn[:], 1, [P(eqn), [C, TB], [2, 4]]),
                        op=ALU.max)
                    eng.tensor_tensor(
                        out=v(f2[:], 0, [P(f2), [2, TB], [1, 2]]),
                        in0=v(f1[:], 0, [P(f1), [4, TB], [2, 2]]),
                        in1=v(f1[:], 1, [P(f1), [4, TB], [2, 2]]),
                        op=ALU.max)
                    eng.tensor_tensor(
                        out=mn[:],
                        in0=v(f2[:], 0, [P(f2), [2, TB]]),
                        in1=v(f2[:], 1, [P(f2), [2, TB]]),
                        op=ALU.max)
                en1 = pool.tile([128, TB], F32, tag="en1")
                eng.tensor_scalar(out=en1[:], in0=mn[:], scalar1=-1.0, scalar2=9.0,
                                  op0=ALU.mult, op1=ALU.add)
                endsel = pool.tile([128, TB], F32, tag="endsel")
                tmp = pool.tile([128, TB], F32, tag="tmpsel")
                for jj in range(C):
                    dst = endsel if jj == 0 else tmp
                    eng.scalar_tensor_tensor(
                        out=dst[:], in0=en1[:], scalar=float(jj + 1),
                        in1=bp1[:, jj * TB:(jj + 1) * TB],
                        op0=ALU.is_equal, op1=ALU.mult)
                    if jj > 0:
                        eng.tensor_tensor(out=endsel[:], in0=endsel[:], in1=tmp[:],
                                          op=ALU.max)
                atm = pool.tile([128, TB], F32, tag="atm")
                eng.tensor_scalar(out=atm[:], in0=tplane[:, r * TB:(r + 1) * TB],
                                  scalar1=lenm1_sb[:], scalar2=None, op0=ALU.is_equal)
                bpt1 = pool.tile([128, TB * C], F32, tag="bpt1")
                dsel = pool.tile([128, TB * C], F32, tag="dsel", bufs=1)
                bp1_tn = v(bp1[:], 0, [P(bp1), [1, TB], [TB, C]])
                eng.tensor_tensor(
                    out=dsel[:],
                    in0=v(endsel[:], 0, [P(endsel), [1, TB], [0, C]]),
                    in1=bp1_tn, op=ALU.subtract)
                eng.tensor_tensor(
                    out=dsel[:],
                    in0=v(dsel[:], 0, [P(dsel), [C, TB], [1, C]]),
                    in1=v(atm[:], 0, [P(atm), [1, TB], [0, C]]),
                    op=ALU.mult)
                eng.tensor_tensor(out=bpt1[:], in0=bp1_tn, in1=dsel[:], op=ALU.add)
                nc.sync.dma_start(out=bp1_d[:, off:off + TB * C], in_=bpt1[:])

        vitpool_cm.__exit__(None, None, None)
        tc.strict_bb_all_engine_barrier()

        # ============ phase 3: chunked backward ============
        with tc.tile_pool(name="ph3", bufs=1) as pool:
            P = lambda t: t[:].ap[0]
            bp_strip = pool.tile([128, STRIP * C], F32)
            nc.sync.dma_start(out=bp_strip[:], in_=bp1_d[:])
            einit1 = pool.tile([128, C * KL], F32)
            nc.sync.dma_start(out=einit1[:], in_=einit1_d)
            cand1 = pool.tile([128, C * KL * S], F32)
            # tl-loop split across DVE / gpsimd by chunk-column range
            P3S = [(vec, 0, KL_DVE)]
            if KL_DVE < KL:
                P3S.append((nc.gpsimd, KL_DVE, KL))
            p3acc = [(eng, k0, k1,
                      pool.tile([128, C * (k1 - k0)], F32, name="acc" + str(k0),
                                tag="acc" + str(k0)),
                      pool.tile([128, C * (k1 - k0)], F32, name="tmp" + str(k0),
                                tag="tmp" + str(k0)))
                     for eng, k0, k1 in P3S]

            def cand_col_r(tl, k0, k1):
                return v(cand1[:], tl + k0 * S,
                         [P(cand1), [KL * S, C], [S, k1 - k0]])

            for tl in range(S - 1, -1, -1):
                for eng, k0, k1, acc, tmp in p3acc:
                    kw = k1 - k0
                    if tl == S - 1:
                        prev = v(einit1[:], k0, [P(einit1), [KL, C], [1, kw]])
                    else:
                        prev = cand_col_r(tl + 1, k0, k1)
                    for j in range(C):
                        dst = acc[:] if j == 0 else tmp[:]
                        eng.scalar_tensor_tensor(
                            out=dst, in0=prev, scalar=float(j + 1),
                            in1=v(bp_strip[:], tl * C + j + k0 * S * C,
                                  [P(bp_strip), [0, C], [S * C, kw]]),
                            op0=ALU.is_equal, op1=ALU.mult)
                        if j > 0:
                            out_ap = cand_col_r(tl, k0, k1) if j == C - 1 else acc[:]
                            eng.tensor_tensor(out=out_ap, in0=acc[:], in1=tmp[:],
                                              op=ALU.max)

            m1a = pool.tile([128, C * NG], F32)
            m1b = pool.tile([128, C * NG], F32)
            t1 = pool.tile([128, C * NG], F32)
            a1 = pool.tile([128, C * NG], F32)
            vec.tensor_copy(out=m1a[:],
                            in_=v(einit1[:], 0, [P(einit1), [KL, C], [G1, NG]]))
            cur, nxt = m1a, m1b
            for kk in range(G1 - 1, -1, -1):
                for j in range(C):
                    dst = a1[:] if j == 0 else t1[:]
                    vec.scalar_tensor_tensor(
                        out=dst, in0=cur[:], scalar=float(j + 1),
                        in1=v(cand1[:], j * KL * S + kk * S,
                              [P(cand1), [0, C], [G1 * S, NG]]),
                        op0=ALU.is_equal, op1=ALU.mult)
                    if j > 0:
                        out_ap = nxt[:] if j == C - 1 else a1[:]
                        vec.tensor_tensor(out=out_ap, in0=a1[:], in1=t1[:],
                                          op=ALU.max)
                cur, nxt = nxt, cur
            m1 = cur

            msa = pool.tile([128, C], F32)
            msb = pool.tile([128, C], F32)
            t2 = pool.tile([128, C], F32)
            a2 = pool.tile([128, C], F32)
            vec.tensor_copy(out=msa[:], in_=v(einit1[:], 0,
                                              [P(einit1), [KL, C], [1, 1]]))
            cur2, nxt2 = msa, msb
            for g in range(NG - 1, -1, -1):
                for j in range(C):
                    dst = a2[:] if j == 0 else t2[:]
                    vec.scalar_tensor_tensor(
                        out=dst, in0=cur2[:], scalar=float(j + 1),
                        in1=v(m1[:], j * NG + g, [P(m1), [0, C], [0, 1]]),
                        op0=ALU.is_equal, op1=ALU.mult)
                    if j > 0:
                        out_ap = nxt2[:] if j == C - 1 else a2[:]
                        vec.tensor_tensor(out=out_ap, in0=a2[:], in1=t2[:],
                                          op=ALU.max)
                cur2, nxt2 = nxt2, cur2
            nc.sync.dma_start(out=smap_d[:], in_=cur2[:])
            tc.strict_bb_all_engine_barrier()

            smap_t = pool.tile([SEQ, NSTRIP * C], F32)
            nc.sync.dma_start(out=smap_t[:],
                              in_=v(smap_d, 0, [[C, SEQ], [C * SEQ, NSTRIP], [1, C]]))
            state = pool.tile([SEQ, 1], F32)
            sacc = pool.tile([SEQ, 1], F32)
            stmp = pool.tile([SEQ, 1], F32)
            estrip = pool.tile([SEQ, NSTRIP], F32)
            vec.memset(state[:], 1.0)
            for sg in range(NSTRIP - 1, -1, -1):
                vec.tensor_copy(out=estrip[:, sg:sg + 1], in_=state[:])
                for j in range(C):
                    dst = sacc if j == 0 else stmp
                    vec.scalar_tensor_tensor(
                        out=dst[:], in0=state[:], scalar=float(j + 1),
                        in1=smap_t[:, sg * C + j:sg * C + j + 1],
                        op0=ALU.is_equal, op1=ALU.mult)
                    if j > 0:
                        out_ap = state[:] if j == C - 1 else sacc[:]
                        vec.tensor_tensor(out=out_ap, in0=sacc[:], in1=stmp[:],
                                          op=ALU.max)
            nc.sync.dma_start(out=estrip_d, in_=estrip[:])
            tc.strict_bb_all_engine_barrier()
            eseed = pool.tile([128, 1], F32)
            nc.sync.dma_start(out=eseed[:],
                              in_=v(estrip_d, 0, [[1, NSTRIP], [NSTRIP, SEQ], [1, 1]]))

            eg = pool.tile([128, NG], F32)
            st2 = pool.tile([128, 1], F32)
            d2a = pool.tile([128, 1], F32)
            d2t = pool.tile([128, 1], F32)
            vec.tensor_copy(out=st2[:], in_=eseed[:])
            for g in range(NG - 1, -1, -1):
                vec.tensor_copy(out=eg[:, g:g + 1], in_=st2[:])
                for j in range(C):
                    dst = d2a if j == 0 else d2t
                    vec.scalar_tensor_tensor(
                        out=dst[:], in0=st2[:], scalar=float(j + 1),
                        in1=v(m1[:], j * NG + g, [P(m1), [0, 1]]),
                        op0=ALU.is_equal, op1=ALU.mult)
                    if j > 0:
                        out_ap = st2[:] if j == C - 1 else d2a[:]
                        vec.tensor_tensor(out=out_ap, in0=d2a[:], in1=d2t[:],
                                          op=ALU.max)

            ek = pool.tile([128, KL], F32)
            st3 = pool.tile([128, NG], F32)
            d1a = pool.tile([128, NG], F32)
            d1t = pool.tile([128, NG], F32)
            vec.tensor_copy(out=st3[:], in_=eg[:])
            for kk in range(G1 - 1, -1, -1):
                vec.tensor_copy(out=v(ek[:], kk, [P(ek), [G1, NG]]), in_=st3[:])
                for j in range(C):
                    dst = d1a if j == 0 else d1t
                    vec.scalar_tensor_tensor(
                        out=dst[:], in0=st3[:], scalar=float(j + 1),
                        in1=v(cand1[:], j * KL * S + kk * S,
                              [P(cand1), [G1 * S, NG]]),
                        op0=ALU.is_equal, op1=ALU.mult)
                    if j > 0:
                        out_ap = st3[:] if j == C - 1 else d1a[:]
                        vec.tensor_tensor(out=out_ap, in0=d1a[:], in1=d1t[:],
                                          op=ALU.max)

            acc2 = pool.tile([128, STRIP], F32)
            tsel = pool.tile([128, STRIP], F32)
            for e in range(C):
                dst = acc2 if e == 0 else tsel
                vec.scalar_tensor_tensor(
                    out=dst[:],
                    in0=v(ek[:], 0, [P(ek), [1, KL], [0, S]]),
                    scalar=float(e + 1),
                    in1=v(cand1[:], e * KL * S, [P(cand1), [S, KL], [1, S]]),
                    op0=ALU.is_equal, op1=ALU.mult)
                if e > 0:
                    vec.tensor_tensor(out=acc2[:], in0=acc2[:], in1=tsel[:],
                                      op=ALU.max)
            mask = pool.tile([128, STRIP], F32)
            vec.tensor_scalar(out=mask[:], in0=tplane[:], scalar1=len_sb[:],
                              scalar2=None, op0=ALU.is_lt)
            vec.tensor_tensor(out=acc2[:], in0=acc2[:], in1=mask[:], op=ALU.mult)
            vec.tensor_scalar(out=acc2[:], in0=acc2[:], scalar1=-1.0,
                              scalar2=None, op0=ALU.add)
            path_i = pool.tile([128, STRIP], I32)
            vec.tensor_copy(out=path_i[:], in_=acc2[:])
            nc.sync.dma_start(out=path_d, in_=path_i[:])


def _build(L):
    import concourse.bacc as bacc
    import concourse.mybir as mybir
    from concourse import tile

    sh = _shapes(L)
    nc = bacc.Bacc("TRN2", target_bir_lowering=False, debug=False,
                   num_devices=NCORES)
    F32 = mybir.dt.float32
    G = sh["G"]
    ins_aps = {
        "obs": nc.dram_tensor("obs", [SEQ, (L + 1) * C], F32, kind="ExternalInput").ap(),
        "obs_lane": nc.dram_tensor("obs_lane", [128, G * (SC + HALO) * C], F32,
                                   kind="ExternalInput").ap(),
        "trep_h": nc.dram_tensor("trep_h", [128, G * C * C], F32,
                                 kind="ExternalInput").ap(),
        "trep": nc.dram_tensor("trep", [128, C * C], F32, kind="ExternalInput").ap(),
        "wdesc": nc.dram_tensor("wdesc", [128, C], F32, kind="ExternalInput").ap(),
        "tplane": nc.dram_tensor("tplane", [128, sh["STRIP"]], F32, kind="ExternalInput").ap(),
        "len_col": nc.dram_tensor("len_col", [128, 1], F32, kind="ExternalInput").ap(),
        "lenm1": nc.dram_tensor("lenm1", [128, 1], F32, kind="ExternalInput").ap(),
        "einit1": nc.dram_tensor("einit1", [128, C * sh["KL"]], F32, kind="ExternalInput").ap(),
    }
    outs_aps = {"path": nc.dram_tensor("path", [128, sh["STRIP"]], mybir.dt.int32,
                                       kind="ExternalOutput").ap()}
    with tile.TileContext(nc) as tc:
        _emit(tc, ins_aps, outs_aps, L)
    nc.compile()
    return nc


def kernel(observes, transitions, lengths):
    from concourse.bass_utils import run_bass_kernel_spmd

    observes = np.asarray(observes, np.float32)
    transitions = np.asarray(transitions, np.float32)
    lengths_np = np.asarray(lengths)
    L = observes.shape[2]

    if L not in _CACHE:
        _CACHE[L] = _build(L)
    nc = _CACHE[L]

    in_maps = [
        _host_prep(observes[SEQ * c:SEQ * (c + 1)], transitions,
                   lengths_np[SEQ * c:SEQ * (c + 1)], L)
        for c in range(NCORES)
    ]
    res = run_bass_kernel_spmd(nc, in_maps, core_ids=list(range(NCORES)))
    out = np.concatenate(
        [_host_post(res.results[c]["path"], L) for c in range(NCORES)], 0)
    return out.astype(np.int32)



# revision 2
# speedup vs baseline: 1.2301x; 1.2301x over previous
"""Batched CRF Viterbi decode (N=64, C=8, L=32768) on 8 TRN2 NeuronCores.

v2: packed-backpointer rewrite of phases 2+3.
  Phase 1 (unchanged): two-pass quantized-replay forward scan -> vit_sb.
  Phase 2: per 128-step round, gpsimd computes d = (fv + T) - vit_bc;
    vec extracts first-argmax backpointers via (d==0)*wdesc max-reduce,
    then packs all 8 next-states' 3-bit backpointers into one fp32 word
    per timestep (base-8, <= 2^24-1 so exact): B_sb [128, STRIP].
    End-node handling is reduced to a masked fv extraction at t==len-1
    (em accumulators) instead of full per-t argmax machinery.
  Phase 3: all backward gathers become 3-op int32 digit extracts
    (shift = 3*idx; digit = (word >> shift) & 7) instead of 15-op
    select loops. Chunk maps / hierarchical composition / final walk
    all operate on packed words; the big cand1 tensor is gone (the
    tl-loop only needs the final tl=0 map; the output walk re-derives
    per-t tags directly from B with the known entry state).
"""
import sys
import numpy as np

if '/opt/trn_rl_repo' not in sys.path:
    sys.path.insert(0, '/opt/trn_rl_repo')

N_FULL, C, L = 64, 8, 32768
SEQ = 8          # sequences per core
NSTRIP = 16      # time strips per core (partition dim = NSTRIP*SEQ = 128)
S = 16           # phase-3 chunk length
NCORES = 8

# phase-1 speculative scan params
SC = 256         # forward chunk length (must be multiple of phase-2 TB)
HALO = 16        # warmup steps per chunk
G_DVE = 8        # chunks per lane handled by the vector engine (rest: gpsimd)

PACK8 = [float(8 ** j) for j in range(8)]
REP8 = 2396745   # 8^0 + 8^1 + ... + 8^7

_CACHE = {}


def _shapes(L):
    STRIP = L // NSTRIP
    TB = min(128, STRIP)
    return dict(STRIP=STRIP, TB=TB, ROUNDS=STRIP // TB, KL=STRIP // S,
                G=STRIP // SC)


def _host_prep(observes_core, transitions, lengths_core, L):
    sh = _shapes(L)
    STRIP, KL, G = sh["STRIP"], sh["KL"], sh["G"]
    obs_t = np.ascontiguousarray(
        np.transpose(np.asarray(observes_core, np.float32), (0, 2, 1)))
    obs_pad = np.concatenate([np.zeros((SEQ, 1, C), np.float32), obs_t], 1)
    T = np.asarray(transitions, np.float32)
    lens = np.asarray(lengths_core).astype(np.float32)
    p = np.arange(128)

    # phase-1 lane obs: lane p=(s,n), chunk g, step j in [0, SC+HALO):
    #   value = obs_t[n, base+j, :] with base = s*STRIP + g*SC - HALO
    #   (zero when base+j < 0; only chunk (s=0,g=0))
    s_idx = p // SEQ
    n_idx = p % SEQ
    j = np.arange(SC + HALO)
    g = np.arange(G)
    tpos = (s_idx[:, None, None] * STRIP + g[None, :, None] * SC
            - HALO + j[None, None, :])          # [128, G, SC+HALO]
    valid = tpos >= 0
    tcl = np.clip(tpos, 0, L - 1)
    obs_lane = obs_t[n_idx[:, None, None], tcl, :]      # [128, G, SC+HALO, C]
    obs_lane = obs_lane * valid[..., None].astype(np.float32)

    # halo transition matrices: identity-ish for the exact first chunk
    trep_h = np.tile(T.reshape(1, 1, C, C), (128, G, 1, 1)).astype(np.float32)
    ident = np.full((C, C), -1e30, np.float32)
    np.fill_diagonal(ident, 0.0)
    trep_h[:SEQ, 0] = ident                     # chunks (s=0, g=0), all seqs

    return {
        "obs": obs_pad.reshape(SEQ, (L + 1) * C),
        "obs_lane": np.ascontiguousarray(obs_lane).reshape(128, G * (SC + HALO) * C),
        "trep_h": np.ascontiguousarray(trep_h).reshape(128, G * C * C),
        "trep": np.tile(T.reshape(1, C * C), (128, 1)).astype(np.float32),
        "wdesc": np.tile((C - np.arange(C, dtype=np.float32)).reshape(1, C), (128, 1)),
        "tplane": ((p[:, None] // SEQ) * STRIP
                   + np.arange(STRIP)[None, :]).astype(np.float32),
        "len_col": lens[p % SEQ][:, None].astype(np.float32),
        "lenm1": (lens[p % SEQ][:, None] - 1.0).astype(np.float32),
        "pow8f": np.tile(np.array(PACK8, np.float32).reshape(1, C), (128, 1)),
        "pow8i": np.tile(np.array(PACK8, np.int32).reshape(1, C), (128, 1)),
        "eci": np.tile(np.repeat(np.arange(C, dtype=np.int32), KL).reshape(1, C * KL),
                       (128, 1)),
    }


def _host_post(path_dev, L):
    STRIP = L // NSTRIP
    return path_dev.reshape(NSTRIP, SEQ, STRIP).transpose(1, 0, 2).reshape(SEQ, L)


def _emit(tc, ins, outs, L):
    import concourse.bass as bass
    import concourse.mybir as mybir
    import bass_rust

    F32 = mybir.dt.float32
    I32 = mybir.dt.int32
    ALU = mybir.AluOpType
    AX = mybir.AxisListType

    def v(ap, off, dims):
        return bass_rust.AP(tensor=ap.tensor, offset=ap.offset + off, ap=dims)

    nc = tc.nc
    sh = _shapes(L)
    STRIP, TB, ROUNDS, KL, G = (sh["STRIP"], sh["TB"], sh["ROUNDS"],
                                sh["KL"], sh["G"])
    G1 = min(8, KL)
    NG = KL // G1
    FLATN = (L + 1) * C
    CH = SC + HALO   # steps per chunk

    obs_d = ins["obs"]
    obs_lane_d = ins["obs_lane"]
    trep_h_d = ins["trep_h"]
    trep_d = ins["trep"]
    wdesc_d = ins["wdesc"]
    tplane_d = ins["tplane"]
    len_d = ins["len_col"]
    lenm1_d = ins["lenm1"]
    pow8f_d = ins["pow8f"]
    pow8i_d = ins["pow8i"]
    eci_d = ins["eci"]
    path_d = outs["path"]

    smap_d = nc.dram_tensor("smap_scratch", [128, 1], I32).ap()
    estrip_d = nc.dram_tensor("estrip_scratch", [SEQ, NSTRIP], I32).ap()
    s0_d = nc.dram_tensor("s0_scratch", [128, G], F32).ap()
    e0_d = nc.dram_tensor("e0_scratch", [128, G], F32).ap()
    r_d = nc.dram_tensor("r_scratch", [SEQ, NSTRIP * G], F32).ap()

    vec = nc.vector
    gps = nc.gpsimd

    with tc.tile_pool(name="const", bufs=1) as cpool:
        trep = cpool.tile([128, C * C], F32)
        wdesc = cpool.tile([128, C], F32)
        tplane = cpool.tile([128, STRIP], F32)
        len_sb = cpool.tile([128, 1], F32)
        lenm1_sb = cpool.tile([128, 1], F32)
        seeds = cpool.tile([128, G * C], F32)
        pow8f = cpool.tile([128, C], F32)
        pow8i = cpool.tile([128, C], I32)
        eci = cpool.tile([128, C * KL], I32)
        B_sb = cpool.tile([128, STRIP], F32)
        em_all = cpool.tile([128, ROUNDS * C], F32)
        nc.sync.dma_start(out=trep[:], in_=trep_d)
        nc.sync.dma_start(out=wdesc[:], in_=wdesc_d)
        nc.sync.dma_start(out=tplane[:], in_=tplane_d)
        nc.sync.dma_start(out=len_sb[:], in_=len_d)
        nc.sync.dma_start(out=lenm1_sb[:], in_=lenm1_d)
        nc.sync.dma_start(out=pow8f[:], in_=pow8f_d)
        nc.sync.dma_start(out=pow8i[:], in_=pow8i_d)
        nc.sync.dma_start(out=eci[:], in_=eci_d)

        # ============ phase 1: two-pass quantized-replay forward scan ============
        K_ALL = NSTRIP * G     # chunks per sequence
        pool_e = nc.gpsimd
        SPLITS = [(vec, 0, G_DVE, "d")]
        if G_DVE < G:
            SPLITS.append((pool_e, G_DVE, G, "p"))
        vitpool_cm = tc.tile_pool(name="vitp", bufs=1)
        vitpool = vitpool_cm.__enter__()
        vit_sb = vitpool.tile([128, STRIP * C], F32)
        with tc.tile_pool(name="ph1c", bufs=1) as ppool:
            obs_lane = ppool.tile([128, G * CH * C], F32)
            trep_h = ppool.tile([128, G * C * C], F32)
            nc.sync.dma_start(out=obs_lane[:], in_=obs_lane_d)
            nc.sync.dma_start(out=trep_h[:], in_=trep_h_d)

            P = lambda t: t[:].ap[0]
            s0 = ppool.tile([128, G], F32)
            e0 = ppool.tile([128, G], F32)

            # per-engine chain state
            st = {}
            for eng, g0, g1, nm in SPLITS:
                ge = g1 - g0
                fv = ppool.tile([128, ge * C], F32, tag="fv" + nm)
                sce = ppool.tile([128, ge * C * C], F32, tag="sc" + nm)
                vtmp = ppool.tile([128, ge * C], F32, tag="vt" + nm)
                st[nm] = dict(
                    eng=eng, g0=g0, g1=g1, ge=ge, fv=fv, sc=sce, vtmp=vtmp,
                    fvb=v(fv[:], 0, [P(fv), [C, ge], [0, C], [1, C]]),
                    treph3=v(trep_h[:], g0 * C * C,
                             [P(trep_h), [C * C, ge], [C, C], [1, C]]),
                    trep3=v(trep[:], 0, [P(trep), [0, ge], [C, C], [1, C]]),
                    sc3=v(sce[:], 0, [P(sce), [C * C, ge], [C, C], [1, C]]),
                    vtmp2=v(vtmp[:], 0, [P(vtmp), [C, ge], [1, C]]),
                )

            def chain(store):
                """Emit one chunked scan pass on both engines. store=False:
                probes only (pass 1). store=True: vit into vit_sb + seeds."""
                for j in range(CH):
                    halo = j < HALO
                    for _, _, _, nm in SPLITS:
                        e = st[nm]
                        eng, ge = e["eng"], e["ge"]
                        eng.tensor_tensor(
                            out=e["sc3"], in0=e["fvb"],
                            in1=(e["treph3"] if halo else e["trep3"]), op=ALU.add)
                        if halo or not store:
                            vcol = e["vtmp2"]
                        else:
                            jr = j - HALO
                            vcol = v(vit_sb[:], (e["g0"] * SC + jr) * C,
                                     [P(vit_sb), [SC * C, ge], [1, C]])
                        eng.tensor_reduce(out=vcol, in_=e["sc3"], axis=AX.X,
                                          op=ALU.max)
                        eng.tensor_tensor(
                            out=e["fv"][:], in0=vcol,
                            in1=v(obs_lane[:], (e["g0"] * CH + j) * C,
                                  [P(obs_lane), [CH * C, ge], [1, C]]),
                            op=ALU.add)
                        if j == HALO - 1:
                            if store:
                                eng.tensor_copy(
                                    out=seeds[:, e["g0"] * C:e["g1"] * C],
                                    in_=e["fv"][:])
                            else:
                                eng.tensor_copy(
                                    out=s0[:, e["g0"]:e["g1"]],
                                    in_=v(e["fv"][:], 0, [P(e["fv"]), [C, ge]]))

            # ---- pass 1: clean chunks from zero; probe frame offsets ----
            for _, _, _, nm in SPLITS:
                st[nm]["eng"].memset(st[nm]["fv"][:], 0.0)
            chain(store=False)
            for _, _, _, nm in SPLITS:
                e = st[nm]
                e["eng"].tensor_copy(out=e0[:, e["g0"]:e["g1"]],
                                     in_=v(e["fv"][:], 0, [P(e["fv"]), [C, e["ge"]]]))
            nc.sync.dma_start(out=e0_d, in_=e0[:])
            nc.sync.dma_start(out=s0_d, in_=s0[:])
            tc.strict_bb_all_engine_barrier()

            # ---- frame offsets: delta -> serial prefix -> snap ----
            # s0_d flat = (s*SEQ+n)*G + g; per-seq view [n, k=s*G+g]
            seq_dims = [[G, SEQ], [SEQ * G, NSTRIP], [1, G]]
            s0_t = ppool.tile([SEQ, K_ALL], F32)
            e0_t = ppool.tile([SEQ, K_ALL], F32)
            nc.sync.dma_start(out=s0_t[:], in_=v(s0_d, 0, seq_dims))
            nc.sync.dma_start(out=e0_t[:], in_=v(e0_d, 0, seq_dims))
            delta = ppool.tile([SEQ, K_ALL], F32)
            vec.memset(delta[:], 0.0)
            vec.tensor_tensor(
                out=delta[:, 0:K_ALL - 1], in0=e0_t[:, 0:K_ALL - 1],
                in1=v(s0_t[:], 1, [P(s0_t), [1, K_ALL - 1]]), op=ALU.subtract)
            r_t = ppool.tile([SEQ, K_ALL], F32)
            vec.memset(r_t[:, 0:1], 0.0)
            for k in range(1, K_ALL):
                vec.tensor_tensor(out=r_t[:, k:k + 1], in0=r_t[:, k - 1:k],
                                  in1=delta[:, k - 1:k], op=ALU.add)
            # snap to the coarse power-of-2 grid (2*ulp at max magnitude)
            vec.tensor_scalar(out=r_t[:], in0=r_t[:], scalar1=98304.0,
                              scalar2=None, op0=ALU.add)
            vec.tensor_scalar(out=r_t[:], in0=r_t[:], scalar1=-98304.0,
                              scalar2=None, op0=ALU.add)
            nc.sync.dma_start(out=r_d, in_=r_t[:])
            tc.strict_bb_all_engine_barrier()

            # ---- pass 2: replay at absolute magnitude ----
            r_sb = ppool.tile([128, G], F32)
            nc.sync.dma_start(
                out=r_sb[:],
                in_=v(r_d, 0, [[G, NSTRIP], [K_ALL, SEQ], [1, G]]))
            for _, _, _, nm in SPLITS:
                e = st[nm]
                e["eng"].tensor_scalar(
                    out=e["fv"][:],
                    in0=v(r_sb[:], e["g0"], [P(r_sb), [1, e["ge"]], [0, C]]),
                    scalar1=0.0, scalar2=None, op0=ALU.add)
            chain(store=True)

        # ============ phase 2: packed backpointer extraction ============
        with tc.tile_pool(name="ph2", bufs=2) as pool:
            P0 = lambda t: t[:].ap[0]
            for r in range(ROUNDS):
                off = r * TB * C
                vbase = (r * TB - 1) * C    # vit_sb col for fv window col 0
                obs_blk = pool.tile([128, (TB + 1) * C], F32, tag="obs")
                fv_blk = pool.tile([128, (TB + 1) * C], F32, tag="fv")
                src_dims = [[STRIP * C, NSTRIP], [FLATN, SEQ], [1, (TB + 1) * C]]
                nc.sync.dma_start(out=obs_blk[:], in_=v(obs_d, off, src_dims))
                if r == 0:
                    # col 0 is seed-replaced; vit_sb has no slot for t=0
                    gps.tensor_tensor(
                        out=fv_blk[:, C:(TB + 1) * C],
                        in0=v(vit_sb[:], 0, [P0(vit_sb), [1, TB * C]]),
                        in1=obs_blk[:, C:(TB + 1) * C], op=ALU.add)
                else:
                    gps.tensor_tensor(
                        out=fv_blk[:],
                        in0=v(vit_sb[:], vbase, [P0(vit_sb), [1, (TB + 1) * C]]),
                        in1=obs_blk[:], op=ALU.add)
                if (r * TB) % SC == 0:
                    gi = (r * TB) // SC
                    gps.tensor_copy(out=fv_blk[:, 0:C],
                                    in_=seeds[:, gi * C:(gi + 1) * C])

                P = lambda t: t[:].ap[0]
                # d[next, t, prev] = (fv[t-1][prev] + T[next,prev]) - vit[t][next]
                sc2 = pool.tile([128, C * TB * C], F32, tag="sc")
                gps.tensor_tensor(
                    out=sc2[:],
                    in0=v(fv_blk[:], 0, [P(fv_blk), [0, C], [C, TB], [1, C]]),
                    in1=v(trep[:], 0, [P(trep), [C, C], [0, TB], [1, C]]),
                    op=ALU.add)
                sc2_3 = v(sc2[:], 0, [P(sc2), [TB * C, C], [C, TB], [1, C]])
                gps.tensor_tensor(
                    out=sc2_3, in0=sc2_3,
                    in1=v(vit_sb[:], vbase + C,
                          [P0(vit_sb), [1, C], [C, TB], [0, C]]),
                    op=ALU.subtract)
                # eqw = (d == 0) * wdesc  (wdesc = 8 - prev -> max picks first)
                # flat 3D APs: STT rejects 4D; [next,t,prev] flat == contiguous
                vec.scalar_tensor_tensor(
                    out=sc2[:], in0=sc2[:], scalar=0.0,
                    in1=v(wdesc[:], 0, [P(wdesc), [0, C * TB], [1, C]]),
                    op0=ALU.is_equal, op1=ALU.mult)
                bpw = pool.tile([128, C * TB], F32, tag="bpw", bufs=1)
                vec.tensor_reduce(out=bpw[:], in_=sc2_3, axis=AX.X, op=ALU.max)
                # bp0 = 8 - bpw  (0-based first-argmax backpointer)
                bp0 = pool.tile([128, C * TB], F32, tag="bp0", bufs=1)
                vec.tensor_scalar(out=bp0[:], in0=bpw[:], scalar1=-1.0,
                                  scalar2=8.0, op0=ALU.mult, op1=ALU.add)
                # pack: B[t] = sum_next bp0[next,t] * 8^next  (<= 2^24-1, exact)
                bp8 = pool.tile([128, C * TB], F32, tag="bp8", bufs=1)
                vec.tensor_tensor(
                    out=bp8[:],
                    in0=v(bp0[:], 0, [P(bp0), [TB, C], [1, TB]]),
                    in1=v(pow8f[:], 0, [P(pow8f), [1, C], [0, TB]]),
                    op=ALU.mult)
                vec.tensor_reduce(
                    out=B_sb[:, r * TB:(r + 1) * TB],
                    in_=v(bp8[:], 0, [P(bp8), [1, TB], [TB, C]]),
                    axis=AX.X, op=ALU.add)

                # end-node accumulator: em_all[:, r*C:] = sum_t atm[t]*fv[t][:]
                atm = pool.tile([128, TB], F32, tag="atm", bufs=1)
                gps.tensor_scalar(out=atm[:], in0=tplane[:, r * TB:(r + 1) * TB],
                                  scalar1=lenm1_sb[:], scalar2=None,
                                  op0=ALU.is_equal)
                emt = pool.tile([128, TB * C], F32, tag="emt")
                gps.tensor_tensor(
                    out=emt[:],
                    in0=v(fv_blk[:], C, [P(fv_blk), [C, TB], [1, C]]),
                    in1=v(atm[:], 0, [P(atm), [1, TB], [0, C]]),
                    op=ALU.mult)
                vec.tensor_reduce(
                    out=em_all[:, r * C:(r + 1) * C],
                    in_=v(emt[:], 0, [P(emt), [1, C], [C, TB]]),
                    axis=AX.X, op=ALU.add)

        vitpool_cm.__exit__(None, None, None)

        # ---- end-node fixup: replace B[len-1] with repunit(end digit) ----
        with tc.tile_pool(name="ph2e", bufs=1) as pool:
            P = lambda t: t[:].ap[0]
            em = pool.tile([128, C], F32)
            vec.tensor_reduce(
                out=em[:],
                in_=v(em_all[:], 0, [P(em_all), [1, C], [C, ROUNDS]]),
                axis=AX.X, op=ALU.add)
            fmax = pool.tile([128, 1], F32)
            vec.tensor_reduce(out=fmax[:], in_=em[:], axis=AX.X, op=ALU.max)
            d2 = pool.tile([128, C], F32)
            vec.tensor_tensor(out=d2[:], in0=em[:],
                              in1=v(fmax[:], 0, [P(fmax), [0, C]]),
                              op=ALU.subtract)
            vec.scalar_tensor_tensor(out=d2[:], in0=d2[:], scalar=0.0,
                                     in1=wdesc[:, 0:C],
                                     op0=ALU.is_equal, op1=ALU.mult)
            w2 = pool.tile([128, 1], F32)
            vec.tensor_reduce(out=w2[:], in_=d2[:], axis=AX.X, op=ALU.max)
            end0 = pool.tile([128, 1], F32)
            vec.tensor_scalar(out=end0[:], in0=w2[:], scalar1=-1.0,
                              scalar2=8.0, op0=ALU.mult, op1=ALU.add)
            # B value at t=len-1 (masked sum; exact since others are 0)
            cmask = pool.tile([128, STRIP], F32)
            vec.tensor_scalar(out=cmask[:], in0=tplane[:], scalar1=lenm1_sb[:],
                              scalar2=None, op0=ALU.is_equal)
            bm = pool.tile([128, STRIP], F32)
            vec.tensor_tensor(out=bm[:], in0=B_sb[:], in1=cmask[:], op=ALU.mult)
            bc = pool.tile([128, 1], F32)
            vec.tensor_reduce(out=bc[:], in_=bm[:], axis=AX.X, op=ALU.add)
            # dg = digit(Bc, end0); rep = dg * REP8 (all 8 digits = dg)
            bci = pool.tile([128, 1], I32)
            e0i = pool.tile([128, 1], I32)
            shx = pool.tile([128, 1], I32)
            dgi = pool.tile([128, 1], I32)
            vec.tensor_copy(out=bci[:], in_=bc[:])
            vec.tensor_copy(out=e0i[:], in_=end0[:])
            vec.tensor_scalar(out=shx[:], in0=e0i[:], scalar1=3, scalar2=None,
                              op0=ALU.mult)
            vec.tensor_tensor(out=dgi[:], in0=bci[:], in1=shx[:],
                              op=ALU.logical_shift_right)
            vec.tensor_scalar(out=dgi[:], in0=dgi[:], scalar1=7, scalar2=None,
                              op0=ALU.bitwise_and)
            vec.tensor_scalar(out=dgi[:], in0=dgi[:], scalar1=REP8,
                              scalar2=None, op0=ALU.mult)
            repf = pool.tile([128, 1], F32)
            vec.tensor_copy(out=repf[:], in_=dgi[:])
            # B += cmask * (rep - B)
            diff = pool.tile([128, STRIP], F32)
            vec.tensor_tensor(out=diff[:],
                              in0=v(repf[:], 0, [P(repf), [0, STRIP]]),
                              in1=B_sb[:], op=ALU.subtract)
            vec.tensor_tensor(out=diff[:], in0=diff[:], in1=cmask[:],
                              op=ALU.mult)
            vec.tensor_tensor(out=B_sb[:], in0=B_sb[:], in1=diff[:], op=ALU.add)

        # ============ phase 3: packed backward ============
        with tc.tile_pool(name="ph3", bufs=1) as pool:
            P = lambda t: t[:].ap[0]
            B_i = pool.tile([128, STRIP], I32)
            vec.tensor_copy(out=B_i[:], in_=B_sb[:])

            # chunk maps: cur[e,k] = tag after traversing chunk k from entry e
            cur = pool.tile([128, C * KL], I32)
            sh1 = pool.tile([128, C * KL], I32)
            gg1 = pool.tile([128, C * KL], I32)
            vec.tensor_copy(out=cur[:], in_=eci[:])
            for tl in range(S - 1, -1, -1):
                vec.tensor_scalar(out=sh1[:], in0=cur[:], scalar1=3,
                                  scalar2=None, op0=ALU.mult)
                vec.tensor_tensor(
                    out=gg1[:],
                    in0=v(B_i[:], tl, [P(B_i), [0, C], [S, KL]]),
                    in1=sh1[:], op=ALU.logical_shift_right)
                vec.tensor_scalar(out=cur[:], in0=gg1[:], scalar1=7,
                                  scalar2=None, op0=ALU.bitwise_and)

            # W1[k] = pack_e(cur)
            w1p = pool.tile([128, KL * C], I32)
            vec.tensor_tensor(
                out=w1p[:],
                in0=v(cur[:], 0, [P(cur), [1, KL], [KL, C]]),
                in1=v(pow8i[:], 0, [P(pow8i), [0, KL], [1, C]]),
                op=ALU.mult)
            W1 = pool.tile([128, KL], I32)
            with nc.allow_low_precision(reason="int32 base-8 pack, exact"):
                vec.tensor_reduce(out=W1[:],
                                  in_=v(w1p[:], 0, [P(w1p), [C, KL], [1, C]]),
                                  axis=AX.X, op=ALU.add)

            # m1[e,g]: compose the G1 chunk maps of each group
            cur1 = pool.tile([128, C * NG], I32)
            sh2 = pool.tile([128, C * NG], I32)
            gg2 = pool.tile([128, C * NG], I32)
            vec.tensor_copy(out=cur1[:],
                            in_=v(eci[:], 0, [P(eci), [KL, C], [1, NG]]))
            for kk in range(G1 - 1, -1, -1):
                vec.tensor_scalar(out=sh2[:], in0=cur1[:], scalar1=3,
                                  scalar2=None, op0=ALU.mult)
                vec.tensor_tensor(
                    out=gg2[:],
                    in0=v(W1[:], kk, [P(W1), [0, C], [G1, NG]]),
                    in1=sh2[:], op=ALU.logical_shift_right)
                vec.tensor_scalar(out=cur1[:], in0=gg2[:], scalar1=7,
                                  scalar2=None, op0=ALU.bitwise_and)

            # Wm[g] = pack_e(m1)
            wmp = pool.tile([128, NG * C], I32)
            vec.tensor_tensor(
                out=wmp[:],
                in0=v(cur1[:], 0, [P(cur1), [1, NG], [NG, C]]),
                in1=v(pow8i[:], 0, [P(pow8i), [0, NG], [1, C]]),
                op=ALU.mult)
            Wm = pool.tile([128, NG], I32)
            with nc.allow_low_precision(reason="int32 base-8 pack, exact"):
                vec.tensor_reduce(out=Wm[:],
                                  in_=v(wmp[:], 0, [P(wmp), [C, NG], [1, C]]),
                                  axis=AX.X, op=ALU.add)

            # smap[e]: compose the NG group maps per (strip, seq) lane
            cur2 = pool.tile([128, C], I32)
            sh3 = pool.tile([128, C], I32)
            gg3 = pool.tile([128, C], I32)
            vec.tensor_copy(out=cur2[:], in_=v(eci[:], 0, [P(eci), [KL, C]]))
            for g in range(NG - 1, -1, -1):
                vec.tensor_scalar(out=sh3[:], in0=cur2[:], scalar1=3,
                                  scalar2=None, op0=ALU.mult)
                vec.tensor_tensor(
                    out=gg3[:],
                    in0=v(Wm[:], g, [P(Wm), [0, C]]),
                    in1=sh3[:], op=ALU.logical_shift_right)
                vec.tensor_scalar(out=cur2[:], in0=gg3[:], scalar1=7,
                                  scalar2=None, op0=ALU.bitwise_and)

            # Wsm = pack_e(smap) -> DRAM -> per-seq strip composition
            wsp = pool.tile([128, C], I32)
            vec.tensor_tensor(out=wsp[:], in0=cur2[:], in1=pow8i[:, 0:C],
                              op=ALU.mult)
            wsm = pool.tile([128, 1], I32)
            with nc.allow_low_precision(reason="int32 base-8 pack, exact"):
                vec.tensor_reduce(out=wsm[:], in_=wsp[:], axis=AX.X, op=ALU.add)
            nc.sync.dma_start(out=smap_d, in_=wsm[:])
            tc.strict_bb_all_engine_barrier()

            wst = pool.tile([SEQ, NSTRIP], I32)
            nc.sync.dma_start(out=wst[:],
                              in_=v(smap_d, 0, [[1, SEQ], [SEQ, NSTRIP], [1, 1]]))
            state = pool.tile([SEQ, 1], I32)
            ssh = pool.tile([SEQ, 1], I32)
            sgg = pool.tile([SEQ, 1], I32)
            estrip = pool.tile([SEQ, NSTRIP], I32)
            vec.memset(state[:], 0)
            for sg in range(NSTRIP - 1, -1, -1):
                vec.tensor_copy(out=estrip[:, sg:sg + 1], in_=state[:])
                vec.tensor_scalar(out=ssh[:], in0=state[:], scalar1=3,
                                  scalar2=None, op0=ALU.mult)
                vec.tensor_tensor(out=sgg[:], in0=wst[:, sg:sg + 1],
                                  in1=ssh[:], op=ALU.logical_shift_right)
                vec.tensor_scalar(out=state[:], in0=sgg[:], scalar1=7,
                                  scalar2=None, op0=ALU.bitwise_and)
            nc.sync.dma_start(out=estrip_d, in_=estrip[:])
            tc.strict_bb_all_engine_barrier()
            eseed = pool.tile([128, 1], I32)
            nc.sync.dma_start(out=eseed[:],
                              in_=v(estrip_d, 0, [[1, NSTRIP], [NSTRIP, SEQ], [1, 1]]))

            # eg[g]: entry state into each group
            eg = pool.tile([128, NG], I32)
            st2 = pool.tile([128, 1], I32)
            esh = pool.tile([128, 1], I32)
            egg = pool.tile([128, 1], I32)
            vec.tensor_copy(out=st2[:], in_=eseed[:])
            for g in range(NG - 1, -1, -1):
                vec.tensor_copy(out=eg[:, g:g + 1], in_=st2[:])
                vec.tensor_scalar(out=esh[:], in0=st2[:], scalar1=3,
                                  scalar2=None, op0=ALU.mult)
                vec.tensor_tensor(out=egg[:], in0=Wm[:, g:g + 1],
                                  in1=esh[:], op=ALU.logical_shift_right)
                vec.tensor_scalar(out=st2[:], in0=egg[:], scalar1=7,
                                  scalar2=None, op0=ALU.bitwise_and)

            # ek[k]: entry state into each chunk
            ek = pool.tile([128, KL], I32)
            st3 = pool.tile([128, NG], I32)
            ksh = pool.tile([128, NG], I32)
            kgg = pool.tile([128, NG], I32)
            vec.tensor_copy(out=st3[:], in_=eg[:])
            for kk in range(G1 - 1, -1, -1):
                vec.tensor_copy(out=v(ek[:], kk, [P(ek), [G1, NG]]), in_=st3[:])
                vec.tensor_scalar(out=ksh[:], in0=st3[:], scalar1=3,
                                  scalar2=None, op0=ALU.mult)
                vec.tensor_tensor(
                    out=kgg[:],
                    in0=v(W1[:], kk, [P(W1), [G1, NG]]),
                    in1=ksh[:], op=ALU.logical_shift_right)
                vec.tensor_scalar(out=st3[:], in0=kgg[:], scalar1=7,
                                  scalar2=None, op0=ALU.bitwise_and)

            # final walk: re-derive per-t tags from B with known entries
            acc2 = pool.tile([128, STRIP], I32)
            stw = pool.tile([128, KL], I32)
            wsh = pool.tile([128, KL], I32)
            vec.tensor_copy(out=stw[:], in_=ek[:])
            for tl in range(S - 1, -1, -1):
                vec.tensor_scalar(out=wsh[:], in0=stw[:], scalar1=3,
                                  scalar2=None, op0=ALU.mult)
                vec.tensor_tensor(
                    out=stw[:],
                    in0=v(B_i[:], tl, [P(B_i), [S, KL]]),
                    in1=wsh[:], op=ALU.logical_shift_right)
                vec.tensor_scalar(out=stw[:], in0=stw[:], scalar1=7,
                                  scalar2=None, op0=ALU.bitwise_and)
                vec.tensor_copy(out=v(acc2[:], tl, [P(acc2), [S, KL]]),
                                in_=stw[:])

            # mask: path = (acc2 + 1) * (t < len) - 1
            maskf = pool.tile([128, STRIP], F32)
            vec.tensor_scalar(out=maskf[:], in0=tplane[:], scalar1=len_sb[:],
                              scalar2=None, op0=ALU.is_lt)
            acc2f = pool.tile([128, STRIP], F32)
            vec.tensor_copy(out=acc2f[:], in_=acc2[:])
            vec.tensor_scalar(out=acc2f[:], in0=acc2f[:], scalar1=1.0,
                              scalar2=None, op0=ALU.add)
            vec.tensor_tensor(out=acc2f[:], in0=acc2f[:], in1=maskf[:],
                              op=ALU.mult)
            vec.tensor_scalar(out=acc2f[:], in0=acc2f[:], scalar1=-1.0,
                              scalar2=None, op0=ALU.add)
            path_i = pool.tile([128, STRIP], I32)
            vec.tensor_copy(out=path_i[:], in_=acc2f[:])
            nc.sync.dma_start(out=path_d, in_=path_i[:])


def _build(L):
    import concourse.bacc as bacc
    import concourse.mybir as mybir
    from concourse import tile

    sh = _shapes(L)
    nc = bacc.Bacc("TRN2", target_bir_lowering=False, debug=False,
                   num_devices=NCORES)
    F32 = mybir.dt.float32
    I32 = mybir.dt.int32
    G = sh["G"]
    ins_aps = {
        "obs": nc.dram_tensor("obs", [SEQ, (L + 1) * C], F32, kind="ExternalInput").ap(),
        "obs_lane": nc.dram_tensor("obs_lane", [128, G * (SC + HALO) * C], F32,
                                   kind="ExternalInput").ap(),
        "trep_h": nc.dram_tensor("trep_h", [128, G * C * C], F32,
                                 kind="ExternalInput").ap(),
        "trep": nc.dram_tensor("trep", [128, C * C], F32, kind="ExternalInput").ap(),
        "wdesc": nc.dram_tensor("wdesc", [128, C], F32, kind="ExternalInput").ap(),
        "tplane": nc.dram_tensor("tplane", [128, sh["STRIP"]], F32, kind="ExternalInput").ap(),
        "len_col": nc.dram_tensor("len_col", [128, 1], F32, kind="ExternalInput").ap(),
        "lenm1": nc.dram_tensor("lenm1", [128, 1], F32, kind="ExternalInput").ap(),
        "pow8f": nc.dram_tensor("pow8f", [128, C], F32, kind="ExternalInput").ap(),
        "pow8i": nc.dram_tensor("pow8i", [128, C], I32, kind="ExternalInput").ap(),
        "eci": nc.dram_tensor("eci", [128, C * sh["KL"]], I32, kind="ExternalInput").ap(),
    }
    outs_aps = {"path": nc.dram_tensor("path", [128, sh["STRIP"]], mybir.dt.int32,
                                       kind="ExternalOutput").ap()}
    with tile.TileContext(nc) as tc:
        _emit(tc, ins_aps, outs_aps, L)
    nc.compile()
    return nc


def kernel(observes, transitions, lengths):
    from concourse.bass_utils import run_bass_kernel_spmd

    observes = np.asarray(observes, np.float32)
    transitions = np.asarray(transitions, np.float32)
    lengths_np = np.asarray(lengths)
    L = observes.shape[2]

    if L not in _CACHE:
        _CACHE[L] = _build(L)
    nc = _CACHE[L]

    in_maps = [
        _host_prep(observes[SEQ * c:SEQ * (c + 1)], transitions,
                   lengths_np[SEQ * c:SEQ * (c + 1)], L)
        for c in range(NCORES)
    ]
    res = run_bass_kernel_spmd(nc, in_maps, core_ids=list(range(NCORES)))
    out = np.concatenate(
        [_host_post(res.results[c]["path"], L) for c in range(NCORES)], 0)
    return out.astype(np.int32)


# revision 3
# speedup vs baseline: 1.2820x; 1.0422x over previous
"""Batched CRF Viterbi decode (N=64, C=8, L=32768) on 8 TRN2 NeuronCores.

v2: packed-backpointer rewrite of phases 2+3.
  Phase 1 (unchanged): two-pass quantized-replay forward scan -> vit_sb.
  Phase 2: per 128-step round, gpsimd computes d = (fv + T) - vit_bc;
    vec extracts first-argmax backpointers via (d==0)*wdesc max-reduce,
    then packs all 8 next-states' 3-bit backpointers into one fp32 word
    per timestep (base-8, <= 2^24-1 so exact): B_sb [128, STRIP].
    End-node handling is reduced to a masked fv extraction at t==len-1
    (em accumulators) instead of full per-t argmax machinery.
  Phase 3: all backward gathers become 3-op int32 digit extracts
    (shift = 3*idx; digit = (word >> shift) & 7) instead of 15-op
    select loops. Chunk maps / hierarchical composition / final walk
    all operate on packed words; the big cand1 tensor is gone (the
    tl-loop only needs the final tl=0 map; the output walk re-derives
    per-t tags directly from B with the known entry state).
"""
import sys
import numpy as np

if '/opt/trn_rl_repo' not in sys.path:
    sys.path.insert(0, '/opt/trn_rl_repo')

N_FULL, C, L = 64, 8, 32768
SEQ = 8          # sequences per core
NSTRIP = 16      # time strips per core (partition dim = NSTRIP*SEQ = 128)
S = 16           # phase-3 chunk length
NCORES = 8

# phase-1 speculative scan params
SC = 128         # forward chunk length (must be multiple of phase-2 TB)
HALO = 16        # warmup steps per chunk
G_DVE = 16       # chunks per lane handled by the vector engine (all: no gps max)

PACK8 = [float(8 ** j) for j in range(8)]
REP8 = 2396745   # 8^0 + 8^1 + ... + 8^7

_CACHE = {}


def _shapes(L):
    STRIP = L // NSTRIP
    TB = min(128, STRIP)
    return dict(STRIP=STRIP, TB=TB, ROUNDS=STRIP // TB, KL=STRIP // S,
                G=STRIP // SC)


def _host_prep(observes_core, transitions, lengths_core, L):
    sh = _shapes(L)
    STRIP, KL, G = sh["STRIP"], sh["KL"], sh["G"]
    obs_t = np.ascontiguousarray(
        np.transpose(np.asarray(observes_core, np.float32), (0, 2, 1)))
    obs_pad = np.concatenate([np.zeros((SEQ, 1, C), np.float32), obs_t], 1)
    T = np.asarray(transitions, np.float32)
    lens = np.asarray(lengths_core).astype(np.float32)
    p = np.arange(128)

    # phase-1 lane obs: lane p=(s,n), chunk g, step j in [0, SC+HALO):
    #   value = obs_t[n, base+j, :] with base = s*STRIP + g*SC - HALO
    #   (zero when base+j < 0; only chunk (s=0,g=0))
    s_idx = p // SEQ
    n_idx = p % SEQ
    j = np.arange(SC + HALO)
    g = np.arange(G)
    tpos = (s_idx[:, None, None] * STRIP + g[None, :, None] * SC
            - HALO + j[None, None, :])          # [128, G, SC+HALO]
    valid = tpos >= 0
    tcl = np.clip(tpos, 0, L - 1)
    obs_lane = obs_t[n_idx[:, None, None], tcl, :]      # [128, G, SC+HALO, C]
    obs_lane = obs_lane * valid[..., None].astype(np.float32)

    # halo transition matrices: identity-ish for the exact first chunk
    trep_h = np.tile(T.reshape(1, 1, C, C), (128, G, 1, 1)).astype(np.float32)
    ident = np.full((C, C), -1e30, np.float32)
    np.fill_diagonal(ident, 0.0)
    trep_h[:SEQ, 0] = ident                     # chunks (s=0, g=0), all seqs

    return {
        "obs": obs_pad.reshape(SEQ, (L + 1) * C),
        "obs_lane": np.ascontiguousarray(obs_lane).reshape(128, G * (SC + HALO) * C),
        "trep_h": np.ascontiguousarray(trep_h).reshape(128, G * C * C),
        "trep": np.tile(T.reshape(1, C * C), (128, 1)).astype(np.float32),
        "wdesc": np.tile((C - np.arange(C, dtype=np.float32)).reshape(1, C), (128, 1)),
        "tplane": ((p[:, None] // SEQ) * STRIP
                   + np.arange(STRIP)[None, :]).astype(np.float32),
        "len_col": lens[p % SEQ][:, None].astype(np.float32),
        "lenm1": (lens[p % SEQ][:, None] - 1.0).astype(np.float32),
        "pow8f": np.tile(np.array(PACK8, np.float32).reshape(1, C), (128, 1)),
        "pow8i": np.tile(np.array(PACK8, np.int32).reshape(1, C), (128, 1)),
        "eci": np.tile(np.repeat(np.arange(C, dtype=np.int32), KL).reshape(1, C * KL),
                       (128, 1)),
    }


def _host_post(path_dev, L):
    STRIP = L // NSTRIP
    return path_dev.reshape(NSTRIP, SEQ, STRIP).transpose(1, 0, 2).reshape(SEQ, L)


def _emit(tc, ins, outs, L):
    import concourse.bass as bass
    import concourse.mybir as mybir
    import bass_rust

    F32 = mybir.dt.float32
    I32 = mybir.dt.int32
    ALU = mybir.AluOpType
    AX = mybir.AxisListType

    def v(ap, off, dims):
        return bass_rust.AP(tensor=ap.tensor, offset=ap.offset + off, ap=dims)

    nc = tc.nc
    sh = _shapes(L)
    STRIP, TB, ROUNDS, KL, G = (sh["STRIP"], sh["TB"], sh["ROUNDS"],
                                sh["KL"], sh["G"])
    G1 = min(8, KL)
    NG = KL // G1
    FLATN = (L + 1) * C
    CH = SC + HALO   # steps per chunk

    obs_d = ins["obs"]
    obs_lane_d = ins["obs_lane"]
    trep_h_d = ins["trep_h"]
    trep_d = ins["trep"]
    wdesc_d = ins["wdesc"]
    tplane_d = ins["tplane"]
    len_d = ins["len_col"]
    lenm1_d = ins["lenm1"]
    pow8f_d = ins["pow8f"]
    pow8i_d = ins["pow8i"]
    eci_d = ins["eci"]
    path_d = outs["path"]

    smap_d = nc.dram_tensor("smap_scratch", [128, 1], I32).ap()
    estrip_d = nc.dram_tensor("estrip_scratch", [SEQ, NSTRIP], I32).ap()
    s0_d = nc.dram_tensor("s0_scratch", [128, G], F32).ap()
    e0_d = nc.dram_tensor("e0_scratch", [128, G], F32).ap()
    r_d = nc.dram_tensor("r_scratch", [SEQ, NSTRIP * G], F32).ap()

    vec = nc.vector
    gps = nc.gpsimd

    with tc.tile_pool(name="const", bufs=1) as cpool:
        trep = cpool.tile([128, C * C], F32)
        wdesc = cpool.tile([128, C], F32)
        tplane = cpool.tile([128, STRIP], F32)
        len_sb = cpool.tile([128, 1], F32)
        lenm1_sb = cpool.tile([128, 1], F32)
        seeds = cpool.tile([128, G * C], F32)
        pow8f = cpool.tile([128, C], F32)
        pow8i = cpool.tile([128, C], I32)
        eci = cpool.tile([128, C * KL], I32)
        B_sb = cpool.tile([128, STRIP], F32)
        em_all = cpool.tile([128, ROUNDS * C], F32)
        nc.sync.dma_start(out=trep[:], in_=trep_d)
        nc.sync.dma_start(out=wdesc[:], in_=wdesc_d)
        nc.sync.dma_start(out=tplane[:], in_=tplane_d)
        nc.sync.dma_start(out=len_sb[:], in_=len_d)
        nc.sync.dma_start(out=lenm1_sb[:], in_=lenm1_d)
        nc.sync.dma_start(out=pow8f[:], in_=pow8f_d)
        nc.sync.dma_start(out=pow8i[:], in_=pow8i_d)
        nc.sync.dma_start(out=eci[:], in_=eci_d)

        # ============ phase 1: two-pass quantized-replay forward scan ============
        K_ALL = NSTRIP * G     # chunks per sequence
        pool_e = nc.gpsimd
        SPLITS = [(vec, 0, G_DVE, "d")]
        if G_DVE < G:
            SPLITS.append((pool_e, G_DVE, G, "p"))
        vitpool_cm = tc.tile_pool(name="vitp", bufs=1)
        vitpool = vitpool_cm.__enter__()
        vit_sb = vitpool.tile([128, STRIP * C], F32)
        with tc.tile_pool(name="ph1c", bufs=1) as ppool:
            obs_lane = ppool.tile([128, G * CH * C], F32)
            trep_h = ppool.tile([128, G * C * C], F32)
            nc.sync.dma_start(out=obs_lane[:], in_=obs_lane_d)
            nc.sync.dma_start(out=trep_h[:], in_=trep_h_d)

            P = lambda t: t[:].ap[0]
            s0 = ppool.tile([128, G], F32)
            e0 = ppool.tile([128, G], F32)

            # per-engine chain state
            st = {}
            for eng, g0, g1, nm in SPLITS:
                ge = g1 - g0
                fv = ppool.tile([128, ge * C], F32, tag="fv" + nm)
                sce = ppool.tile([128, ge * C * C], F32, tag="sc" + nm)
                vtmp = ppool.tile([128, ge * C], F32, tag="vt" + nm)
                f1 = ppool.tile([128, ge * C * 4], F32, tag="f1" + nm)
                f2 = ppool.tile([128, ge * C * 2], F32, tag="f2" + nm)
                st[nm] = dict(
                    eng=eng, g0=g0, g1=g1, ge=ge, fv=fv, sc=sce, vtmp=vtmp,
                    f1=f1, f2=f2,
                    fvb=v(fv[:], 0, [P(fv), [C, ge], [0, C], [1, C]]),
                    treph3=v(trep_h[:], g0 * C * C,
                             [P(trep_h), [C * C, ge], [C, C], [1, C]]),
                    trep3=v(trep[:], 0, [P(trep), [0, ge], [C, C], [1, C]]),
                    sc3=v(sce[:], 0, [P(sce), [C * C, ge], [C, C], [1, C]]),
                    vtmp2=v(vtmp[:], 0, [P(vtmp), [C, ge], [1, C]]),
                )

            def chain(store):
                """Emit one chunked scan pass on both engines. store=False:
                probes only (pass 1). store=True: vit into vit_sb + seeds."""
                for j in range(CH):
                    halo = j < HALO
                    for _, _, _, nm in SPLITS:
                        e = st[nm]
                        eng, ge = e["eng"], e["ge"]
                        eng.tensor_tensor(
                            out=e["sc3"], in0=e["fvb"],
                            in1=(e["treph3"] if halo else e["trep3"]), op=ALU.add)
                        if halo or not store:
                            vcol = e["vtmp2"]
                        else:
                            jr = j - HALO
                            vcol = v(vit_sb[:], (e["g0"] * SC + jr) * C,
                                     [P(vit_sb), [SC * C, ge], [1, C]])
                        if eng is vec:
                            eng.tensor_reduce(out=vcol, in_=e["sc3"], axis=AX.X,
                                              op=ALU.max)
                        else:
                            # gpsimd: no free-axis reduce -> log-tree max
                            sce, f1, f2 = e["sc"], e["f1"], e["f2"]
                            eng.tensor_tensor(
                                out=f1[:],
                                in0=v(sce[:], 0,
                                      [P(sce), [C * C, ge], [C, C], [2, 4]]),
                                in1=v(sce[:], 1,
                                      [P(sce), [C * C, ge], [C, C], [2, 4]]),
                                op=ALU.max)
                            eng.tensor_tensor(
                                out=f2[:],
                                in0=v(f1[:], 0,
                                      [P(f1), [C * 4, ge], [4, C], [2, 2]]),
                                in1=v(f1[:], 1,
                                      [P(f1), [C * 4, ge], [4, C], [2, 2]]),
                                op=ALU.max)
                            eng.tensor_tensor(
                                out=vcol,
                                in0=v(f2[:], 0,
                                      [P(f2), [C * 2, ge], [2, C]]),
                                in1=v(f2[:], 1,
                                      [P(f2), [C * 2, ge], [2, C]]),
                                op=ALU.max)
                        eng.tensor_tensor(
                            out=e["fv"][:], in0=vcol,
                            in1=v(obs_lane[:], (e["g0"] * CH + j) * C,
                                  [P(obs_lane), [CH * C, ge], [1, C]]),
                            op=ALU.add)
                        if j == HALO - 1:
                            if store:
                                eng.tensor_copy(
                                    out=seeds[:, e["g0"] * C:e["g1"] * C],
                                    in_=e["fv"][:])
                            else:
                                eng.tensor_copy(
                                    out=s0[:, e["g0"]:e["g1"]],
                                    in_=v(e["fv"][:], 0, [P(e["fv"]), [C, ge]]))

            # ---- pass 1: clean chunks from zero; probe frame offsets ----
            for _, _, _, nm in SPLITS:
                st[nm]["eng"].memset(st[nm]["fv"][:], 0.0)
            chain(store=False)
            for _, _, _, nm in SPLITS:
                e = st[nm]
                e["eng"].tensor_copy(out=e0[:, e["g0"]:e["g1"]],
                                     in_=v(e["fv"][:], 0, [P(e["fv"]), [C, e["ge"]]]))
            nc.sync.dma_start(out=e0_d, in_=e0[:])
            nc.sync.dma_start(out=s0_d, in_=s0[:])
            tc.strict_bb_all_engine_barrier()

            # ---- frame offsets: delta -> serial prefix -> snap ----
            # s0_d flat = (s*SEQ+n)*G + g; per-seq view [n, k=s*G+g]
            seq_dims = [[G, SEQ], [SEQ * G, NSTRIP], [1, G]]
            s0_t = ppool.tile([SEQ, K_ALL], F32)
            e0_t = ppool.tile([SEQ, K_ALL], F32)
            nc.sync.dma_start(out=s0_t[:], in_=v(s0_d, 0, seq_dims))
            nc.sync.dma_start(out=e0_t[:], in_=v(e0_d, 0, seq_dims))
            delta = ppool.tile([SEQ, K_ALL], F32)
            vec.memset(delta[:], 0.0)
            vec.tensor_tensor(
                out=delta[:, 0:K_ALL - 1], in0=e0_t[:, 0:K_ALL - 1],
                in1=v(s0_t[:], 1, [P(s0_t), [1, K_ALL - 1]]), op=ALU.subtract)
            # snap deltas to the coarse 2^-7 grid FIRST: every partial sum of
            # snapped deltas is exactly representable, so the log-depth tree
            # prefix below is bit-identical to a sequential prefix.
            vec.tensor_scalar(out=delta[:], in0=delta[:], scalar1=98304.0,
                              scalar2=None, op0=ALU.add)
            vec.tensor_scalar(out=delta[:], in0=delta[:], scalar1=-98304.0,
                              scalar2=None, op0=ALU.add)
            xa = ppool.tile([SEQ, K_ALL], F32)
            xb = ppool.tile([SEQ, K_ALL], F32)
            vec.memset(xa[:, 0:1], 0.0)
            vec.tensor_copy(out=xa[:, 1:K_ALL], in_=delta[:, 0:K_ALL - 1])
            sstep = 1
            while sstep < K_ALL:
                vec.tensor_copy(out=xb[:, 0:sstep], in_=xa[:, 0:sstep])
                vec.tensor_tensor(out=xb[:, sstep:K_ALL],
                                  in0=xa[:, sstep:K_ALL],
                                  in1=xa[:, 0:K_ALL - sstep], op=ALU.add)
                xa, xb = xb, xa
                sstep *= 2
            # r values are exact multiples of 2^-7 already; final snap no-op
            nc.sync.dma_start(out=r_d, in_=xa[:])
            tc.strict_bb_all_engine_barrier()

            # ---- pass 2: replay at absolute magnitude ----
            r_sb = ppool.tile([128, G], F32)
            nc.sync.dma_start(
                out=r_sb[:],
                in_=v(r_d, 0, [[G, NSTRIP], [K_ALL, SEQ], [1, G]]))
            for _, _, _, nm in SPLITS:
                e = st[nm]
                e["eng"].tensor_scalar(
                    out=e["fv"][:],
                    in0=v(r_sb[:], e["g0"], [P(r_sb), [1, e["ge"]], [0, C]]),
                    scalar1=0.0, scalar2=None, op0=ALU.add)
            chain(store=True)

        # ============ phase 2: packed backpointer extraction ============
        with tc.tile_pool(name="ph2", bufs=2) as pool:
            P0 = lambda t: t[:].ap[0]
            for r in range(ROUNDS):
                off = r * TB * C
                vbase = (r * TB - 1) * C    # vit_sb col for fv window col 0
                obs_blk = pool.tile([128, (TB + 1) * C], F32, tag="obs")
                fv_blk = pool.tile([128, (TB + 1) * C], F32, tag="fv")
                src_dims = [[STRIP * C, NSTRIP], [FLATN, SEQ], [1, (TB + 1) * C]]
                nc.sync.dma_start(out=obs_blk[:], in_=v(obs_d, off, src_dims))
                if r == 0:
                    # col 0 is seed-replaced; vit_sb has no slot for t=0
                    gps.tensor_tensor(
                        out=fv_blk[:, C:(TB + 1) * C],
                        in0=v(vit_sb[:], 0, [P0(vit_sb), [1, TB * C]]),
                        in1=obs_blk[:, C:(TB + 1) * C], op=ALU.add)
                else:
                    gps.tensor_tensor(
                        out=fv_blk[:],
                        in0=v(vit_sb[:], vbase, [P0(vit_sb), [1, (TB + 1) * C]]),
                        in1=obs_blk[:], op=ALU.add)
                if (r * TB) % SC == 0:
                    gi = (r * TB) // SC
                    gps.tensor_copy(out=fv_blk[:, 0:C],
                                    in_=seeds[:, gi * C:(gi + 1) * C])

                P = lambda t: t[:].ap[0]
                # sc2[next, t, prev] = fv[t-1][prev] + T[next,prev] on the Act
                # engine: 64 bias-add Identity calls (bias = per-partition
                # scalar AP from trep). Act has its own SBUF ports, so this
                # runs truly parallel to vec/gps.
                sc2 = pool.tile([128, C * TB * C], F32, tag="sc")
                for n_ in range(C):
                    for p_ in range(C):
                        nc.scalar.activation(
                            out=v(sc2[:], n_ * TB * C + p_, [P(sc2), [C, TB]]),
                            in_=v(fv_blk[:], p_, [P(fv_blk), [C, TB]]),
                            func=mybir.ActivationFunctionType.Identity,
                            bias=trep[:, n_ * C + p_:n_ * C + p_ + 1],
                            scale=1.0)
                sc2_3 = v(sc2[:], 0, [P(sc2), [TB * C, C], [C, TB], [1, C]])
                gps.tensor_tensor(
                    out=sc2_3, in0=sc2_3,
                    in1=v(vit_sb[:], vbase + C,
                          [P0(vit_sb), [1, C], [C, TB], [0, C]]),
                    op=ALU.subtract)
                # eqw = (d == 0) * wdesc  (wdesc = 8 - prev -> max picks first)
                # flat 3D APs: STT rejects 4D; [next,t,prev] flat == contiguous
                vec.scalar_tensor_tensor(
                    out=sc2[:], in0=sc2[:], scalar=0.0,
                    in1=v(wdesc[:], 0, [P(wdesc), [0, C * TB], [1, C]]),
                    op0=ALU.is_equal, op1=ALU.mult)
                bpw = pool.tile([128, C * TB], F32, tag="bpw", bufs=1)
                vec.tensor_reduce(out=bpw[:], in_=sc2_3, axis=AX.X, op=ALU.max)
                # bp0 = 8 - bpw  (0-based first-argmax backpointer)
                bp0 = pool.tile([128, C * TB], F32, tag="bp0", bufs=1)
                vec.tensor_scalar(out=bp0[:], in0=bpw[:], scalar1=-1.0,
                                  scalar2=8.0, op0=ALU.mult, op1=ALU.add)
                # pack: B[t] = sum_next bp0[next,t] * 8^next  (<= 2^24-1, exact)
                bp8 = pool.tile([128, C * TB], F32, tag="bp8", bufs=1)
                vec.tensor_tensor(
                    out=bp8[:],
                    in0=v(bp0[:], 0, [P(bp0), [TB, C], [1, TB]]),
                    in1=v(pow8f[:], 0, [P(pow8f), [1, C], [0, TB]]),
                    op=ALU.mult)
                vec.tensor_reduce(
                    out=B_sb[:, r * TB:(r + 1) * TB],
                    in_=v(bp8[:], 0, [P(bp8), [1, TB], [TB, C]]),
                    axis=AX.X, op=ALU.add)

                # end-node accumulator: em_all[:, r*C:] = sum_t atm[t]*fv[t][:]
                atm = pool.tile([128, TB], F32, tag="atm", bufs=1)
                vec.tensor_scalar(out=atm[:], in0=tplane[:, r * TB:(r + 1) * TB],
                                  scalar1=lenm1_sb[:], scalar2=None,
                                  op0=ALU.is_equal)
                emt = pool.tile([128, TB * C], F32, tag="emt")
                gps.tensor_tensor(
                    out=emt[:],
                    in0=v(fv_blk[:], C, [P(fv_blk), [C, TB], [1, C]]),
                    in1=v(atm[:], 0, [P(atm), [1, TB], [0, C]]),
                    op=ALU.mult)
                vec.tensor_reduce(
                    out=em_all[:, r * C:(r + 1) * C],
                    in_=v(emt[:], 0, [P(emt), [1, C], [C, TB]]),
                    axis=AX.X, op=ALU.add)

        vitpool_cm.__exit__(None, None, None)

        # ---- end-node fixup: replace B[len-1] with repunit(end digit) ----
        with tc.tile_pool(name="ph2e", bufs=1) as pool:
            P = lambda t: t[:].ap[0]
            em = pool.tile([128, C], F32)
            vec.tensor_reduce(
                out=em[:],
                in_=v(em_all[:], 0, [P(em_all), [1, C], [C, ROUNDS]]),
                axis=AX.X, op=ALU.add)
            fmax = pool.tile([128, 1], F32)
            vec.tensor_reduce(out=fmax[:], in_=em[:], axis=AX.X, op=ALU.max)
            d2 = pool.tile([128, C], F32)
            vec.tensor_tensor(out=d2[:], in0=em[:],
                              in1=v(fmax[:], 0, [P(fmax), [0, C]]),
                              op=ALU.subtract)
            vec.scalar_tensor_tensor(out=d2[:], in0=d2[:], scalar=0.0,
                                     in1=wdesc[:, 0:C],
                                     op0=ALU.is_equal, op1=ALU.mult)
            w2 = pool.tile([128, 1], F32)
            vec.tensor_reduce(out=w2[:], in_=d2[:], axis=AX.X, op=ALU.max)
            end0 = pool.tile([128, 1], F32)
            vec.tensor_scalar(out=end0[:], in0=w2[:], scalar1=-1.0,
                              scalar2=8.0, op0=ALU.mult, op1=ALU.add)
            # B value at t=len-1 (masked sum; exact since others are 0)
            cmask = pool.tile([128, STRIP], F32)
            vec.tensor_scalar(out=cmask[:], in0=tplane[:], scalar1=lenm1_sb[:],
                              scalar2=None, op0=ALU.is_equal)
            bm = pool.tile([128, STRIP], F32)
            vec.tensor_tensor(out=bm[:], in0=B_sb[:], in1=cmask[:], op=ALU.mult)
            bc = pool.tile([128, 1], F32)
            vec.tensor_reduce(out=bc[:], in_=bm[:], axis=AX.X, op=ALU.add)
            # dg = digit(Bc, end0); rep = dg * REP8 (all 8 digits = dg)
            bci = pool.tile([128, 1], I32)
            e0i = pool.tile([128, 1], I32)
            shx = pool.tile([128, 1], I32)
            dgi = pool.tile([128, 1], I32)
            vec.tensor_copy(out=bci[:], in_=bc[:])
            vec.tensor_copy(out=e0i[:], in_=end0[:])
            vec.tensor_scalar(out=shx[:], in0=e0i[:], scalar1=3, scalar2=None,
                              op0=ALU.mult)
            vec.tensor_tensor(out=dgi[:], in0=bci[:], in1=shx[:],
                              op=ALU.logical_shift_right)
            vec.tensor_scalar(out=dgi[:], in0=dgi[:], scalar1=7, scalar2=None,
                              op0=ALU.bitwise_and)
            vec.tensor_scalar(out=dgi[:], in0=dgi[:], scalar1=REP8,
                              scalar2=None, op0=ALU.mult)
            repf = pool.tile([128, 1], F32)
            vec.tensor_copy(out=repf[:], in_=dgi[:])
            # B += cmask * (rep - B)
            diff = pool.tile([128, STRIP], F32)
            vec.tensor_tensor(out=diff[:],
                              in0=v(repf[:], 0, [P(repf), [0, STRIP]]),
                              in1=B_sb[:], op=ALU.subtract)
            vec.tensor_tensor(out=diff[:], in0=diff[:], in1=cmask[:],
                              op=ALU.mult)
            vec.tensor_tensor(out=B_sb[:], in0=B_sb[:], in1=diff[:], op=ALU.add)

        # ============ phase 3: packed backward ============
        with tc.tile_pool(name="ph3", bufs=1) as pool:
            P = lambda t: t[:].ap[0]
            B_i = pool.tile([128, STRIP], I32)
            vec.tensor_copy(out=B_i[:], in_=B_sb[:])

            # chunk maps: cur[e,k] = tag after traversing chunk k from entry e
            cur = pool.tile([128, C * KL], I32)
            sh1 = pool.tile([128, C * KL], I32)
            gg1 = pool.tile([128, C * KL], I32)
            vec.tensor_copy(out=cur[:], in_=eci[:])
            for tl in range(S - 1, -1, -1):
                vec.tensor_scalar(out=sh1[:], in0=cur[:], scalar1=3,
                                  scalar2=None, op0=ALU.mult)
                vec.tensor_tensor(
                    out=gg1[:],
                    in0=v(B_i[:], tl, [P(B_i), [0, C], [S, KL]]),
                    in1=sh1[:], op=ALU.logical_shift_right)
                vec.tensor_scalar(out=cur[:], in0=gg1[:], scalar1=7,
                                  scalar2=None, op0=ALU.bitwise_and)

            # W1[k] = pack_e(cur)
            w1p = pool.tile([128, KL * C], I32)
            vec.tensor_tensor(
                out=w1p[:],
                in0=v(cur[:], 0, [P(cur), [1, KL], [KL, C]]),
                in1=v(pow8i[:], 0, [P(pow8i), [0, KL], [1, C]]),
                op=ALU.mult)
            W1 = pool.tile([128, KL], I32)
            with nc.allow_low_precision(reason="int32 base-8 pack, exact"):
                vec.tensor_reduce(out=W1[:],
                                  in_=v(w1p[:], 0, [P(w1p), [C, KL], [1, C]]),
                                  axis=AX.X, op=ALU.add)

            # m1[e,g]: compose the G1 chunk maps of each group
            cur1 = pool.tile([128, C * NG], I32)
            sh2 = pool.tile([128, C * NG], I32)
            gg2 = pool.tile([128, C * NG], I32)
            vec.tensor_copy(out=cur1[:],
                            in_=v(eci[:], 0, [P(eci), [KL, C], [1, NG]]))
            for kk in range(G1 - 1, -1, -1):
                vec.tensor_scalar(out=sh2[:], in0=cur1[:], scalar1=3,
                                  scalar2=None, op0=ALU.mult)
                vec.tensor_tensor(
                    out=gg2[:],
                    in0=v(W1[:], kk, [P(W1), [0, C], [G1, NG]]),
                    in1=sh2[:], op=ALU.logical_shift_right)
                vec.tensor_scalar(out=cur1[:], in0=gg2[:], scalar1=7,
                                  scalar2=None, op0=ALU.bitwise_and)

            # Wm[g] = pack_e(m1)
            wmp = pool.tile([128, NG * C], I32)
            vec.tensor_tensor(
                out=wmp[:],
                in0=v(cur1[:], 0, [P(cur1), [1, NG], [NG, C]]),
                in1=v(pow8i[:], 0, [P(pow8i), [0, NG], [1, C]]),
                op=ALU.mult)
            Wm = pool.tile([128, NG], I32)
            with nc.allow_low_precision(reason="int32 base-8 pack, exact"):
                vec.tensor_reduce(out=Wm[:],
                                  in_=v(wmp[:], 0, [P(wmp), [C, NG], [1, C]]),
                                  axis=AX.X, op=ALU.add)

            # smap[e]: compose the NG group maps per (strip, seq) lane
            cur2 = pool.tile([128, C], I32)
            sh3 = pool.tile([128, C], I32)
            gg3 = pool.tile([128, C], I32)
            vec.tensor_copy(out=cur2[:], in_=v(eci[:], 0, [P(eci), [KL, C]]))
            for g in range(NG - 1, -1, -1):
                vec.tensor_scalar(out=sh3[:], in0=cur2[:], scalar1=3,
                                  scalar2=None, op0=ALU.mult)
                vec.tensor_tensor(
                    out=gg3[:],
                    in0=v(Wm[:], g, [P(Wm), [0, C]]),
                    in1=sh3[:], op=ALU.logical_shift_right)
                vec.tensor_scalar(out=cur2[:], in0=gg3[:], scalar1=7,
                                  scalar2=None, op0=ALU.bitwise_and)

            # Wsm = pack_e(smap) -> DRAM -> per-seq strip composition
            wsp = pool.tile([128, C], I32)
            vec.tensor_tensor(out=wsp[:], in0=cur2[:], in1=pow8i[:, 0:C],
                              op=ALU.mult)
            wsm = pool.tile([128, 1], I32)
            with nc.allow_low_precision(reason="int32 base-8 pack, exact"):
                vec.tensor_reduce(out=wsm[:], in_=wsp[:], axis=AX.X, op=ALU.add)
            nc.sync.dma_start(out=smap_d, in_=wsm[:])
            tc.strict_bb_all_engine_barrier()

            wst = pool.tile([SEQ, NSTRIP], I32)
            nc.sync.dma_start(out=wst[:],
                              in_=v(smap_d, 0, [[1, SEQ], [SEQ, NSTRIP], [1, 1]]))
            state = pool.tile([SEQ, 1], I32)
            ssh = pool.tile([SEQ, 1], I32)
            sgg = pool.tile([SEQ, 1], I32)
            estrip = pool.tile([SEQ, NSTRIP], I32)
            vec.memset(state[:], 0)
            for sg in range(NSTRIP - 1, -1, -1):
                vec.tensor_copy(out=estrip[:, sg:sg + 1], in_=state[:])
                vec.tensor_scalar(out=ssh[:], in0=state[:], scalar1=3,
                                  scalar2=None, op0=ALU.mult)
                vec.tensor_tensor(out=sgg[:], in0=wst[:, sg:sg + 1],
                                  in1=ssh[:], op=ALU.logical_shift_right)
                vec.tensor_scalar(out=state[:], in0=sgg[:], scalar1=7,
                                  scalar2=None, op0=ALU.bitwise_and)
            nc.sync.dma_start(out=estrip_d, in_=estrip[:])
            tc.strict_bb_all_engine_barrier()
            eseed = pool.tile([128, 1], I32)
            nc.sync.dma_start(out=eseed[:],
                              in_=v(estrip_d, 0, [[1, NSTRIP], [NSTRIP, SEQ], [1, 1]]))

            # eg[g]: entry state into each group
            eg = pool.tile([128, NG], I32)
            st2 = pool.tile([128, 1], I32)
            esh = pool.tile([128, 1], I32)
            egg = pool.tile([128, 1], I32)
            vec.tensor_copy(out=st2[:], in_=eseed[:])
            for g in range(NG - 1, -1, -1):
                vec.tensor_copy(out=eg[:, g:g + 1], in_=st2[:])
                vec.tensor_scalar(out=esh[:], in0=st2[:], scalar1=3,
                                  scalar2=None, op0=ALU.mult)
                vec.tensor_tensor(out=egg[:], in0=Wm[:, g:g + 1],
                                  in1=esh[:], op=ALU.logical_shift_right)
                vec.tensor_scalar(out=st2[:], in0=egg[:], scalar1=7,
                                  scalar2=None, op0=ALU.bitwise_and)

            # ek[k]: entry state into each chunk
            ek = pool.tile([128, KL], I32)
            st3 = pool.tile([128, NG], I32)
            ksh = pool.tile([128, NG], I32)
            kgg = pool.tile([128, NG], I32)
            vec.tensor_copy(out=st3[:], in_=eg[:])
            for kk in range(G1 - 1, -1, -1):
                vec.tensor_copy(out=v(ek[:], kk, [P(ek), [G1, NG]]), in_=st3[:])
                vec.tensor_scalar(out=ksh[:], in0=st3[:], scalar1=3,
                                  scalar2=None, op0=ALU.mult)
                vec.tensor_tensor(
                    out=kgg[:],
                    in0=v(W1[:], kk, [P(W1), [G1, NG]]),
                    in1=ksh[:], op=ALU.logical_shift_right)
                vec.tensor_scalar(out=st3[:], in0=kgg[:], scalar1=7,
                                  scalar2=None, op0=ALU.bitwise_and)

            # final walk: re-derive per-t tags from B with known entries
            acc2 = pool.tile([128, STRIP], I32)
            stw = pool.tile([128, KL], I32)
            wsh = pool.tile([128, KL], I32)
            vec.tensor_copy(out=stw[:], in_=ek[:])
            for tl in range(S - 1, -1, -1):
                vec.tensor_scalar(out=wsh[:], in0=stw[:], scalar1=3,
                                  scalar2=None, op0=ALU.mult)
                vec.tensor_tensor(
                    out=stw[:],
                    in0=v(B_i[:], tl, [P(B_i), [S, KL]]),
                    in1=wsh[:], op=ALU.logical_shift_right)
                vec.tensor_scalar(out=stw[:], in0=stw[:], scalar1=7,
                                  scalar2=None, op0=ALU.bitwise_and)
                vec.tensor_copy(out=v(acc2[:], tl, [P(acc2), [S, KL]]),
                                in_=stw[:])

            # mask: path = (acc2 + 1) * (t < len) - 1
            maskf = pool.tile([128, STRIP], F32)
            vec.tensor_scalar(out=maskf[:], in0=tplane[:], scalar1=len_sb[:],
                              scalar2=None, op0=ALU.is_lt)
            acc2f = pool.tile([128, STRIP], F32)
            vec.tensor_copy(out=acc2f[:], in_=acc2[:])
            vec.tensor_scalar(out=acc2f[:], in0=acc2f[:], scalar1=1.0,
                              scalar2=None, op0=ALU.add)
            vec.tensor_tensor(out=acc2f[:], in0=acc2f[:], in1=maskf[:],
                              op=ALU.mult)
            vec.tensor_scalar(out=acc2f[:], in0=acc2f[:], scalar1=-1.0,
                              scalar2=None, op0=ALU.add)
            path_i = pool.tile([128, STRIP], I32)
            vec.tensor_copy(out=path_i[:], in_=acc2f[:])
            nc.sync.dma_start(out=path_d, in_=path_i[:])


def _build(L):
    import concourse.bacc as bacc
    import concourse.mybir as mybir
    from concourse import tile

    sh = _shapes(L)
    nc = bacc.Bacc("TRN2", target_bir_lowering=False, debug=False,
                   num_devices=NCORES)
    F32 = mybir.dt.float32
    I32 = mybir.dt.int32
    G = sh["G"]
    ins_aps = {
        "obs": nc.dram_tensor("obs", [SEQ, (L + 1) * C], F32, kind="ExternalInput").ap(),
        "obs_lane": nc.dram_tensor("obs_lane", [128, G * (SC + HALO) * C], F32,
                                   kind="ExternalInput").ap(),
        "trep_h": nc.dram_tensor("trep_h", [128, G * C * C], F32,
                                 kind="ExternalInput").ap(),
        "trep": nc.dram_tensor("trep", [128, C * C], F32, kind="ExternalInput").ap(),
        "wdesc": nc.dram_tensor("wdesc", [128, C], F32, kind="ExternalInput").ap(),
        "tplane": nc.dram_tensor("tplane", [128, sh["STRIP"]], F32, kind="ExternalInput").ap(),
        "len_col": nc.dram_tensor("len_col", [128, 1], F32, kind="ExternalInput").ap(),
        "lenm1": nc.dram_tensor("lenm1", [128, 1], F32, kind="ExternalInput").ap(),
        "pow8f": nc.dram_tensor("pow8f", [128, C], F32, kind="ExternalInput").ap(),
        "pow8i": nc.dram_tensor("pow8i", [128, C], I32, kind="ExternalInput").ap(),
        "eci": nc.dram_tensor("eci", [128, C * sh["KL"]], I32, kind="ExternalInput").ap(),
    }
    outs_aps = {"path": nc.dram_tensor("path", [128, sh["STRIP"]], mybir.dt.int32,
                                       kind="ExternalOutput").ap()}
    with tile.TileContext(nc) as tc:
        _emit(tc, ins_aps, outs_aps, L)
    nc.compile()
    return nc


def kernel(observes, transitions, lengths):
    from concourse.bass_utils import run_bass_kernel_spmd

    observes = np.asarray(observes, np.float32)
    transitions = np.asarray(transitions, np.float32)
    lengths_np = np.asarray(lengths)
    L = observes.shape[2]

    if L not in _CACHE:
        _CACHE[L] = _build(L)
    nc = _CACHE[L]

    in_maps = [
        _host_prep(observes[SEQ * c:SEQ * (c + 1)], transitions,
                   lengths_np[SEQ * c:SEQ * (c + 1)], L)
        for c in range(NCORES)
    ]
    res = run_bass_kernel_spmd(nc, in_maps, core_ids=list(range(NCORES)))
    out = np.concatenate(
        [_host_post(res.results[c]["path"], L) for c in range(NCORES)], 0)
    return out.astype(np.int32)


# revision 4
# speedup vs baseline: 1.3105x; 1.0222x over previous
"""Batched CRF Viterbi decode (N=64, C=8, L=32768) on 8 TRN2 NeuronCores.

v2: packed-backpointer rewrite of phases 2+3.
  Phase 1 (unchanged): two-pass quantized-replay forward scan -> vit_sb.
  Phase 2: per 128-step round, gpsimd computes d = (fv + T) - vit_bc;
    vec extracts first-argmax backpointers via (d==0)*wdesc max-reduce,
    then packs all 8 next-states' 3-bit backpointers into one fp32 word
    per timestep (base-8, <= 2^24-1 so exact): B_sb [128, STRIP].
    End-node handling is reduced to a masked fv extraction at t==len-1
    (em accumulators) instead of full per-t argmax machinery.
  Phase 3: all backward gathers become 3-op int32 digit extracts
    (shift = 3*idx; digit = (word >> shift) & 7) instead of 15-op
    select loops. Chunk maps / hierarchical composition / final walk
    all operate on packed words; the big cand1 tensor is gone (the
    tl-loop only needs the final tl=0 map; the output walk re-derives
    per-t tags directly from B with the known entry state).
"""
import sys
import numpy as np

if '/opt/trn_rl_repo' not in sys.path:
    sys.path.insert(0, '/opt/trn_rl_repo')

N_FULL, C, L = 64, 8, 32768
SEQ = 8          # sequences per core
NSTRIP = 16      # time strips per core (partition dim = NSTRIP*SEQ = 128)
S = 16           # phase-3 chunk length
NCORES = 8

# phase-1 speculative scan params
SC = 128         # forward chunk length (must be multiple of phase-2 TB)
HALO = 16        # warmup steps per chunk
G_DVE = 16       # chunks per lane handled by the vector engine (all: no gps max)

PACK8 = [float(8 ** j) for j in range(8)]
REP8 = 2396745   # 8^0 + 8^1 + ... + 8^7

_CACHE = {}


def _shapes(L):
    STRIP = L // NSTRIP
    TB = min(128, STRIP)
    return dict(STRIP=STRIP, TB=TB, ROUNDS=STRIP // TB, KL=STRIP // S,
                G=STRIP // SC)


def _host_prep(observes_core, transitions, lengths_core, L):
    sh = _shapes(L)
    STRIP, KL, G = sh["STRIP"], sh["KL"], sh["G"]
    obs_t = np.ascontiguousarray(
        np.transpose(np.asarray(observes_core, np.float32), (0, 2, 1)))
    obs_pad = np.concatenate([np.zeros((SEQ, 1, C), np.float32), obs_t], 1)
    T = np.asarray(transitions, np.float32)
    lens = np.asarray(lengths_core).astype(np.float32)
    p = np.arange(128)

    # phase-1 lane obs: lane p=(s,n), chunk g, step j in [0, SC+HALO):
    #   value = obs_t[n, base+j, :] with base = s*STRIP + g*SC - HALO
    #   (zero when base+j < 0; only chunk (s=0,g=0))
    s_idx = p // SEQ
    n_idx = p % SEQ
    j = np.arange(SC + HALO)
    g = np.arange(G)
    tpos = (s_idx[:, None, None] * STRIP + g[None, :, None] * SC
            - HALO + j[None, None, :])          # [128, G, SC+HALO]
    valid = tpos >= 0
    tcl = np.clip(tpos, 0, L - 1)
    obs_lane = obs_t[n_idx[:, None, None], tcl, :]      # [128, G, SC+HALO, C]
    obs_lane = obs_lane * valid[..., None].astype(np.float32)

    # halo transition matrices: identity-ish for the exact first chunk
    trep_h = np.tile(T.reshape(1, 1, C, C), (128, G, 1, 1)).astype(np.float32)
    ident = np.full((C, C), -1e30, np.float32)
    np.fill_diagonal(ident, 0.0)
    trep_h[:SEQ, 0] = ident                     # chunks (s=0, g=0), all seqs

    return {
        "obs": obs_pad.reshape(SEQ, (L + 1) * C),
        "obs_lane": np.ascontiguousarray(obs_lane).reshape(128, G * (SC + HALO) * C),
        "trep_h": np.ascontiguousarray(trep_h).reshape(128, G * C * C),
        "trep": np.tile(T.reshape(1, C * C), (128, 1)).astype(np.float32),
        "wdesc": np.tile((C - np.arange(C, dtype=np.float32)).reshape(1, C), (128, 1)),
        "tplane": ((p[:, None] // SEQ) * STRIP
                   + np.arange(STRIP)[None, :]).astype(np.float32),
        "len_col": lens[p % SEQ][:, None].astype(np.float32),
        "lenm1": (lens[p % SEQ][:, None] - 1.0).astype(np.float32),
        "pow8f": np.tile(np.array(PACK8, np.float32).reshape(1, C), (128, 1)),
        "pow8nf": np.tile(-np.array(PACK8, np.float32).reshape(1, C), (128, 1)),
        "pow8i": np.tile(np.array(PACK8, np.int32).reshape(1, C), (128, 1)),
        "eci": np.tile(np.repeat(np.arange(C, dtype=np.int32), KL).reshape(1, C * KL),
                       (128, 1)),
        # fp16 copies for the probe pass (pass 1 only needs approximate drift)
        "obs_lane16": np.ascontiguousarray(obs_lane).reshape(
            128, G * (SC + HALO) * C).astype(np.float16),
        "trep16": np.tile(T.reshape(1, C * C), (128, 1)).astype(np.float16),
        "trep_h16": np.clip(trep_h, -30000.0, None).reshape(
            128, G * C * C).astype(np.float16),
    }


def _host_post(path_dev, L):
    STRIP = L // NSTRIP
    return path_dev.reshape(NSTRIP, SEQ, STRIP).transpose(1, 0, 2).reshape(SEQ, L)


def _emit(tc, ins, outs, L):
    import concourse.bass as bass
    import concourse.mybir as mybir
    import bass_rust

    F32 = mybir.dt.float32
    I32 = mybir.dt.int32
    ALU = mybir.AluOpType
    AX = mybir.AxisListType

    def v(ap, off, dims):
        return bass_rust.AP(tensor=ap.tensor, offset=ap.offset + off, ap=dims)

    nc = tc.nc
    sh = _shapes(L)
    STRIP, TB, ROUNDS, KL, G = (sh["STRIP"], sh["TB"], sh["ROUNDS"],
                                sh["KL"], sh["G"])
    G1 = min(8, KL)
    NG = KL // G1
    FLATN = (L + 1) * C
    CH = SC + HALO   # steps per chunk

    F16 = mybir.dt.float16
    obs_d = ins["obs"]
    obs_lane_d = ins["obs_lane"]
    trep_h_d = ins["trep_h"]
    trep_d = ins["trep"]
    wdesc_d = ins["wdesc"]
    tplane_d = ins["tplane"]
    len_d = ins["len_col"]
    lenm1_d = ins["lenm1"]
    pow8f_d = ins["pow8f"]
    pow8nf_d = ins["pow8nf"]
    pow8i_d = ins["pow8i"]
    eci_d = ins["eci"]
    obs_lane16_d = ins["obs_lane16"]
    trep16_d = ins["trep16"]
    trep_h16_d = ins["trep_h16"]
    path_d = outs["path"]

    smap_d = nc.dram_tensor("smap_scratch", [128, 1], I32).ap()
    estrip_d = nc.dram_tensor("estrip_scratch", [SEQ, NSTRIP], I32).ap()
    s0_d = nc.dram_tensor("s0_scratch", [128, G], F32).ap()
    e0_d = nc.dram_tensor("e0_scratch", [128, G], F32).ap()
    r_d = nc.dram_tensor("r_scratch", [SEQ, NSTRIP * G], F32).ap()

    vec = nc.vector
    gps = nc.gpsimd

    with tc.tile_pool(name="const", bufs=1) as cpool:
        trep = cpool.tile([128, C * C], F32)
        len_sb = cpool.tile([128, 1], F32)
        lenm1_sb = cpool.tile([128, 1], F32)
        seeds = cpool.tile([128, G * C], F32)
        nc.sync.dma_start(out=trep[:], in_=trep_d)
        nc.sync.dma_start(out=len_sb[:], in_=len_d)
        nc.sync.dma_start(out=lenm1_sb[:], in_=lenm1_d)

        # ============ phase 1: two-pass quantized-replay forward scan ============
        # pass 1 (probe) runs in fp16: it only measures per-chunk drift, which
        # is snapped to the coarse 2^-7 grid anyway; fp16 error (<=~8 over the
        # whole prefix) stays well inside the shift-invariance tolerance.
        # pass 2 (replay) is bit-exact fp32.
        K_ALL = NSTRIP * G     # chunks per sequence
        vitpool_cm = tc.tile_pool(name="vitp", bufs=1)
        vitpool = vitpool_cm.__enter__()
        vit_sb = vitpool.tile([128, STRIP * C], F32)
        with tc.tile_pool(name="ph1c", bufs=1) as ppool:
            obs_lane = ppool.tile([128, G * CH * C], F32)
            trep_h = ppool.tile([128, G * C * C], F32)
            obs_lane16 = ppool.tile([128, G * CH * C], F16)
            trep16 = ppool.tile([128, C * C], F16)
            trep_h16 = ppool.tile([128, G * C * C], F16)
            nc.sync.dma_start(out=obs_lane[:], in_=obs_lane_d)
            nc.sync.dma_start(out=trep_h[:], in_=trep_h_d)
            nc.sync.dma_start(out=obs_lane16[:], in_=obs_lane16_d)
            nc.sync.dma_start(out=trep16[:], in_=trep16_d)
            nc.sync.dma_start(out=trep_h16[:], in_=trep_h16_d)

            P = lambda t: t[:].ap[0]
            s0 = ppool.tile([128, G], F32)
            e0 = ppool.tile([128, G], F32)

            def mk_state(dt, nm, obs_t, trep_t, treph_t):
                fv = ppool.tile([128, G * C], dt, tag="fv" + nm)
                sce = ppool.tile([128, G * C * C], dt, tag="sc" + nm)
                vtmp = ppool.tile([128, G * C], dt, tag="vt" + nm)
                return dict(
                    fv=fv, sc=sce, vtmp=vtmp, obs=obs_t,
                    fvb=v(fv[:], 0, [P(fv), [C, G], [0, C], [1, C]]),
                    treph3=v(treph_t[:], 0,
                             [P(treph_t), [C * C, G], [C, C], [1, C]]),
                    trep3=v(trep_t[:], 0, [P(trep_t), [0, G], [C, C], [1, C]]),
                    sc3=v(sce[:], 0, [P(sce), [C * C, G], [C, C], [1, C]]),
                    vtmp2=v(vtmp[:], 0, [P(vtmp), [C, G], [1, C]]),
                )

            st16 = mk_state(F16, "h", obs_lane16, trep16, trep_h16)
            st32 = mk_state(F32, "f", obs_lane, trep, trep_h)

            def chain(e, store):
                """One chunked scan pass on the vector engine. store=False:
                probes only (pass 1). store=True: vit into vit_sb + seeds."""
                for j in range(CH):
                    halo = j < HALO
                    vec.tensor_tensor(
                        out=e["sc3"], in0=e["fvb"],
                        in1=(e["treph3"] if halo else e["trep3"]), op=ALU.add)
                    if halo or not store:
                        vcol = e["vtmp2"]
                    else:
                        jr = j - HALO
                        vcol = v(vit_sb[:], jr * C,
                                 [P(vit_sb), [SC * C, G], [1, C]])
                    vec.tensor_reduce(out=vcol, in_=e["sc3"], axis=AX.X,
                                      op=ALU.max)
                    vec.tensor_tensor(
                        out=e["fv"][:], in0=vcol,
                        in1=v(e["obs"][:], j * C,
                              [P(e["obs"]), [CH * C, G], [1, C]]),
                        op=ALU.add)
                    if j == HALO - 1:
                        if store:
                            vec.tensor_copy(out=seeds[:], in_=e["fv"][:])
                        else:
                            vec.tensor_copy(
                                out=s0[:],
                                in_=v(e["fv"][:], 0, [P(e["fv"]), [C, G]]))

            # ---- pass 1 (fp16): clean chunks from zero; probe offsets ----
            vec.memset(st16["fv"][:], 0.0)
            chain(st16, store=False)
            vec.tensor_copy(out=e0[:],
                            in_=v(st16["fv"][:], 0, [P(st16["fv"]), [C, G]]))
            nc.sync.dma_start(out=e0_d, in_=e0[:])
            nc.sync.dma_start(out=s0_d, in_=s0[:])
            tc.strict_bb_all_engine_barrier()

            # ---- frame offsets: delta -> serial prefix -> snap ----
            # s0_d flat = (s*SEQ+n)*G + g; per-seq view [n, k=s*G+g]
            seq_dims = [[G, SEQ], [SEQ * G, NSTRIP], [1, G]]
            s0_t = ppool.tile([SEQ, K_ALL], F32)
            e0_t = ppool.tile([SEQ, K_ALL], F32)
            nc.sync.dma_start(out=s0_t[:], in_=v(s0_d, 0, seq_dims))
            nc.sync.dma_start(out=e0_t[:], in_=v(e0_d, 0, seq_dims))
            delta = ppool.tile([SEQ, K_ALL], F32)
            vec.memset(delta[:], 0.0)
            vec.tensor_tensor(
                out=delta[:, 0:K_ALL - 1], in0=e0_t[:, 0:K_ALL - 1],
                in1=v(s0_t[:], 1, [P(s0_t), [1, K_ALL - 1]]), op=ALU.subtract)
            # snap deltas to the coarse 2^-7 grid FIRST: every partial sum of
            # snapped deltas is exactly representable, so the log-depth tree
            # prefix below is bit-identical to a sequential prefix.
            vec.tensor_scalar(out=delta[:], in0=delta[:], scalar1=98304.0,
                              scalar2=None, op0=ALU.add)
            vec.tensor_scalar(out=delta[:], in0=delta[:], scalar1=-98304.0,
                              scalar2=None, op0=ALU.add)
            xa = ppool.tile([SEQ, K_ALL], F32)
            xb = ppool.tile([SEQ, K_ALL], F32)
            vec.memset(xa[:, 0:1], 0.0)
            vec.tensor_copy(out=xa[:, 1:K_ALL], in_=delta[:, 0:K_ALL - 1])
            sstep = 1
            while sstep < K_ALL:
                vec.tensor_copy(out=xb[:, 0:sstep], in_=xa[:, 0:sstep])
                vec.tensor_tensor(out=xb[:, sstep:K_ALL],
                                  in0=xa[:, sstep:K_ALL],
                                  in1=xa[:, 0:K_ALL - sstep], op=ALU.add)
                xa, xb = xb, xa
                sstep *= 2
            # r values are exact multiples of 2^-7 already; final snap no-op
            nc.sync.dma_start(out=r_d, in_=xa[:])
            tc.strict_bb_all_engine_barrier()

            # ---- pass 2 (fp32): replay at absolute magnitude ----
            r_sb = ppool.tile([128, G], F32)
            nc.sync.dma_start(
                out=r_sb[:],
                in_=v(r_d, 0, [[G, NSTRIP], [K_ALL, SEQ], [1, G]]))
            vec.tensor_scalar(
                out=st32["fv"][:],
                in0=v(r_sb[:], 0, [P(r_sb), [1, G], [0, C]]),
                scalar1=0.0, scalar2=None, op0=ALU.add)
            chain(st32, store=True)

        # consts used only by phases 2+3 (allocated after phase 1's pool is
        # freed so the fp16 probe copies fit in SBUF)
        cpool2_cm = tc.tile_pool(name="const2", bufs=1)
        cpool2 = cpool2_cm.__enter__()
        wdesc = cpool2.tile([128, C], F32)
        tplane = cpool2.tile([128, STRIP], F32)
        pow8f = cpool2.tile([128, C], F32)
        pow8nf = cpool2.tile([128, C], F32)
        pow8i = cpool2.tile([128, C], I32)
        eci = cpool2.tile([128, C * KL], I32)
        B_sb = cpool2.tile([128, STRIP], F32)
        em_all = cpool2.tile([128, ROUNDS * C], F32)
        nc.sync.dma_start(out=wdesc[:], in_=wdesc_d)
        nc.sync.dma_start(out=tplane[:], in_=tplane_d)
        nc.sync.dma_start(out=pow8f[:], in_=pow8f_d)
        nc.sync.dma_start(out=pow8nf[:], in_=pow8nf_d)
        nc.sync.dma_start(out=pow8i[:], in_=pow8i_d)
        nc.sync.dma_start(out=eci[:], in_=eci_d)

        # ============ phase 2: packed backpointer extraction ============
        with tc.tile_pool(name="ph2", bufs=2) as pool:
            P0 = lambda t: t[:].ap[0]
            for r in range(ROUNDS):
                off = r * TB * C
                vbase = (r * TB - 1) * C    # vit_sb col for fv window col 0
                obs_blk = pool.tile([128, (TB + 1) * C], F32, tag="obs")
                fv_blk = pool.tile([128, (TB + 1) * C], F32, tag="fv")
                src_dims = [[STRIP * C, NSTRIP], [FLATN, SEQ], [1, (TB + 1) * C]]
                nc.sync.dma_start(out=obs_blk[:], in_=v(obs_d, off, src_dims))
                if r == 0:
                    # col 0 is seed-replaced; vit_sb has no slot for t=0
                    gps.tensor_tensor(
                        out=fv_blk[:, C:(TB + 1) * C],
                        in0=v(vit_sb[:], 0, [P0(vit_sb), [1, TB * C]]),
                        in1=obs_blk[:, C:(TB + 1) * C], op=ALU.add)
                else:
                    gps.tensor_tensor(
                        out=fv_blk[:],
                        in0=v(vit_sb[:], vbase, [P0(vit_sb), [1, (TB + 1) * C]]),
                        in1=obs_blk[:], op=ALU.add)
                if (r * TB) % SC == 0:
                    gi = (r * TB) // SC
                    gps.tensor_copy(out=fv_blk[:, 0:C],
                                    in_=seeds[:, gi * C:(gi + 1) * C])

                P = lambda t: t[:].ap[0]
                # sc2[next, t, prev] = fv[t-1][prev] + T[next,prev] on the Act
                # engine: 64 bias-add Identity calls (bias = per-partition
                # scalar AP from trep). Act has its own SBUF ports, so this
                # runs truly parallel to vec/gps.
                sc2 = pool.tile([128, C * TB * C], F32, tag="sc")
                for n_ in range(C):
                    for p_ in range(C):
                        nc.scalar.activation(
                            out=v(sc2[:], n_ * TB * C + p_, [P(sc2), [C, TB]]),
                            in_=v(fv_blk[:], p_, [P(fv_blk), [C, TB]]),
                            func=mybir.ActivationFunctionType.Identity,
                            bias=trep[:, n_ * C + p_:n_ * C + p_ + 1],
                            scale=1.0)
                sc2_3 = v(sc2[:], 0, [P(sc2), [TB * C, C], [C, TB], [1, C]])
                gps.tensor_tensor(
                    out=sc2_3, in0=sc2_3,
                    in1=v(vit_sb[:], vbase + C,
                          [P0(vit_sb), [1, C], [C, TB], [0, C]]),
                    op=ALU.subtract)
                # eqw = (d == 0) * wdesc  (wdesc = 8 - prev -> max picks first)
                # flat 3D APs: STT rejects 4D; [next,t,prev] flat == contiguous
                vec.scalar_tensor_tensor(
                    out=sc2[:], in0=sc2[:], scalar=0.0,
                    in1=v(wdesc[:], 0, [P(wdesc), [0, C * TB], [1, C]]),
                    op0=ALU.is_equal, op1=ALU.mult)
                bpw = pool.tile([128, C * TB], F32, tag="bpw", bufs=1)
                vec.tensor_reduce(out=bpw[:], in_=sc2_3, axis=AX.X, op=ALU.max)
                # fused pack step: bp8[next,t] = (bpw - 8) * (-8^next)
                #                = (8 - bpw) * 8^next = bp0 * 8^next
                # (B[t] = sum_next bp0*8^next <= 2^24-1, every partial exact)
                bp8 = pool.tile([128, C * TB], F32, tag="bp8", bufs=1)
                vec.scalar_tensor_tensor(
                    out=v(bp8[:], 0, [P(bp8), [TB, C], [1, TB]]),
                    in0=v(bpw[:], 0, [P(bpw), [TB, C], [1, TB]]),
                    scalar=8.0,
                    in1=v(pow8nf[:], 0, [P(pow8nf), [1, C], [0, TB]]),
                    op0=ALU.subtract, op1=ALU.mult)
                vec.tensor_reduce(
                    out=B_sb[:, r * TB:(r + 1) * TB],
                    in_=v(bp8[:], 0, [P(bp8), [1, TB], [TB, C]]),
                    axis=AX.X, op=ALU.add)

                # end-node accumulator: em_all[:, r*C:] = sum_t atm[t]*fv[t][:]
                atm = pool.tile([128, TB], F32, tag="atm", bufs=1)
                vec.tensor_scalar(out=atm[:], in0=tplane[:, r * TB:(r + 1) * TB],
                                  scalar1=lenm1_sb[:], scalar2=None,
                                  op0=ALU.is_equal)
                emt = pool.tile([128, TB * C], F32, tag="emt")
                gps.tensor_tensor(
                    out=emt[:],
                    in0=v(fv_blk[:], C, [P(fv_blk), [C, TB], [1, C]]),
                    in1=v(atm[:], 0, [P(atm), [1, TB], [0, C]]),
                    op=ALU.mult)
                vec.tensor_reduce(
                    out=em_all[:, r * C:(r + 1) * C],
                    in_=v(emt[:], 0, [P(emt), [1, C], [C, TB]]),
                    axis=AX.X, op=ALU.add)

        # ---- end-node fixup: replace B[len-1] with repunit(end digit) ----
        with tc.tile_pool(name="ph2e", bufs=1) as pool:
            P = lambda t: t[:].ap[0]
            em = pool.tile([128, C], F32)
            vec.tensor_reduce(
                out=em[:],
                in_=v(em_all[:], 0, [P(em_all), [1, C], [C, ROUNDS]]),
                axis=AX.X, op=ALU.add)
            fmax = pool.tile([128, 1], F32)
            vec.tensor_reduce(out=fmax[:], in_=em[:], axis=AX.X, op=ALU.max)
            d2 = pool.tile([128, C], F32)
            vec.tensor_tensor(out=d2[:], in0=em[:],
                              in1=v(fmax[:], 0, [P(fmax), [0, C]]),
                              op=ALU.subtract)
            vec.scalar_tensor_tensor(out=d2[:], in0=d2[:], scalar=0.0,
                                     in1=wdesc[:, 0:C],
                                     op0=ALU.is_equal, op1=ALU.mult)
            w2 = pool.tile([128, 1], F32)
            vec.tensor_reduce(out=w2[:], in_=d2[:], axis=AX.X, op=ALU.max)
            end0 = pool.tile([128, 1], F32)
            vec.tensor_scalar(out=end0[:], in0=w2[:], scalar1=-1.0,
                              scalar2=8.0, op0=ALU.mult, op1=ALU.add)
            # B value at t=len-1 (masked sum; exact since others are 0)
            cmask = pool.tile([128, STRIP], F32)
            vec.tensor_scalar(out=cmask[:], in0=tplane[:], scalar1=lenm1_sb[:],
                              scalar2=None, op0=ALU.is_equal)
            bm = pool.tile([128, STRIP], F32)
            vec.tensor_tensor(out=bm[:], in0=B_sb[:], in1=cmask[:], op=ALU.mult)
            bc = pool.tile([128, 1], F32)
            vec.tensor_reduce(out=bc[:], in_=bm[:], axis=AX.X, op=ALU.add)
            # dg = digit(Bc, end0); rep = dg * REP8 (all 8 digits = dg)
            bci = pool.tile([128, 1], I32)
            e0i = pool.tile([128, 1], I32)
            shx = pool.tile([128, 1], I32)
            dgi = pool.tile([128, 1], I32)
            vec.tensor_copy(out=bci[:], in_=bc[:])
            vec.tensor_copy(out=e0i[:], in_=end0[:])
            vec.tensor_scalar(out=shx[:], in0=e0i[:], scalar1=3, scalar2=None,
                              op0=ALU.mult)
            vec.tensor_tensor(out=dgi[:], in0=bci[:], in1=shx[:],
                              op=ALU.logical_shift_right)
            vec.tensor_scalar(out=dgi[:], in0=dgi[:], scalar1=7, scalar2=None,
                              op0=ALU.bitwise_and)
            vec.tensor_scalar(out=dgi[:], in0=dgi[:], scalar1=REP8,
                              scalar2=None, op0=ALU.mult)
            repf = pool.tile([128, 1], F32)
            vec.tensor_copy(out=repf[:], in_=dgi[:])
            # B += cmask * (rep - B)
            diff = pool.tile([128, STRIP], F32)
            vec.tensor_tensor(out=diff[:],
                              in0=v(repf[:], 0, [P(repf), [0, STRIP]]),
                              in1=B_sb[:], op=ALU.subtract)
            vec.tensor_tensor(out=diff[:], in0=diff[:], in1=cmask[:],
                              op=ALU.mult)
            vec.tensor_tensor(out=B_sb[:], in0=B_sb[:], in1=diff[:], op=ALU.add)

        # ============ phase 3: packed backward ============
        with tc.tile_pool(name="ph3", bufs=1) as pool:
            P = lambda t: t[:].ap[0]
            B_i = pool.tile([128, STRIP], I32)
            vec.tensor_copy(out=B_i[:], in_=B_sb[:])

            # chunk maps: cur[e,k] = tag after traversing chunk k from entry e
            cur = pool.tile([128, C * KL], I32)
            sh1 = pool.tile([128, C * KL], I32)
            gg1 = pool.tile([128, C * KL], I32)
            vec.tensor_copy(out=cur[:], in_=eci[:])
            for tl in range(S - 1, -1, -1):
                vec.tensor_scalar(out=sh1[:], in0=cur[:], scalar1=3,
                                  scalar2=None, op0=ALU.mult)
                vec.tensor_tensor(
                    out=gg1[:],
                    in0=v(B_i[:], tl, [P(B_i), [0, C], [S, KL]]),
                    in1=sh1[:], op=ALU.logical_shift_right)
                vec.tensor_scalar(out=cur[:], in0=gg1[:], scalar1=7,
                                  scalar2=None, op0=ALU.bitwise_and)

            # W1[k] = pack_e(cur)
            w1p = pool.tile([128, KL * C], I32)
            vec.tensor_tensor(
                out=w1p[:],
                in0=v(cur[:], 0, [P(cur), [1, KL], [KL, C]]),
                in1=v(pow8i[:], 0, [P(pow8i), [0, KL], [1, C]]),
                op=ALU.mult)
            W1 = pool.tile([128, KL], I32)
            with nc.allow_low_precision(reason="int32 base-8 pack, exact"):
                vec.tensor_reduce(out=W1[:],
                                  in_=v(w1p[:], 0, [P(w1p), [C, KL], [1, C]]),
                                  axis=AX.X, op=ALU.add)

            # m1[e,g]: compose the G1 chunk maps of each group
            cur1 = pool.tile([128, C * NG], I32)
            sh2 = pool.tile([128, C * NG], I32)
            gg2 = pool.tile([128, C * NG], I32)
            vec.tensor_copy(out=cur1[:],
                            in_=v(eci[:], 0, [P(eci), [KL, C], [1, NG]]))
            for kk in range(G1 - 1, -1, -1):
                vec.tensor_scalar(out=sh2[:], in0=cur1[:], scalar1=3,
                                  scalar2=None, op0=ALU.mult)
                vec.tensor_tensor(
                    out=gg2[:],
                    in0=v(W1[:], kk, [P(W1), [0, C], [G1, NG]]),
                    in1=sh2[:], op=ALU.logical_shift_right)
                vec.tensor_scalar(out=cur1[:], in0=gg2[:], scalar1=7,
                                  scalar2=None, op0=ALU.bitwise_and)

            # Wm[g] = pack_e(m1)
            wmp = pool.tile([128, NG * C], I32)
            vec.tensor_tensor(
                out=wmp[:],
                in0=v(cur1[:], 0, [P(cur1), [1, NG], [NG, C]]),
                in1=v(pow8i[:], 0, [P(pow8i), [0, NG], [1, C]]),
                op=ALU.mult)
            Wm = pool.tile([128, NG], I32)
            with nc.allow_low_precision(reason="int32 base-8 pack, exact"):
                vec.tensor_reduce(out=Wm[:],
                                  in_=v(wmp[:], 0, [P(wmp), [C, NG], [1, C]]),
                                  axis=AX.X, op=ALU.add)

            # smap[e]: compose the NG group maps per (strip, seq) lane
            cur2 = pool.tile([128, C], I32)
            sh3 = pool.tile([128, C], I32)
            gg3 = pool.tile([128, C], I32)
            vec.tensor_copy(out=cur2[:], in_=v(eci[:], 0, [P(eci), [KL, C]]))
            for g in range(NG - 1, -1, -1):
                vec.tensor_scalar(out=sh3[:], in0=cur2[:], scalar1=3,
                                  scalar2=None, op0=ALU.mult)
                vec.tensor_tensor(
                    out=gg3[:],
                    in0=v(Wm[:], g, [P(Wm), [0, C]]),
                    in1=sh3[:], op=ALU.logical_shift_right)
                vec.tensor_scalar(out=cur2[:], in0=gg3[:], scalar1=7,
                                  scalar2=None, op0=ALU.bitwise_and)

            # Wsm = pack_e(smap) -> DRAM -> per-seq strip composition
            wsp = pool.tile([128, C], I32)
            vec.tensor_tensor(out=wsp[:], in0=cur2[:], in1=pow8i[:, 0:C],
                              op=ALU.mult)
            wsm = pool.tile([128, 1], I32)
            with nc.allow_low_precision(reason="int32 base-8 pack, exact"):
                vec.tensor_reduce(out=wsm[:], in_=wsp[:], axis=AX.X, op=ALU.add)
            nc.sync.dma_start(out=smap_d, in_=wsm[:])
            tc.strict_bb_all_engine_barrier()

            wst = pool.tile([SEQ, NSTRIP], I32)
            nc.sync.dma_start(out=wst[:],
                              in_=v(smap_d, 0, [[1, SEQ], [SEQ, NSTRIP], [1, 1]]))
            state = pool.tile([SEQ, 1], I32)
            ssh = pool.tile([SEQ, 1], I32)
            sgg = pool.tile([SEQ, 1], I32)
            estrip = pool.tile([SEQ, NSTRIP], I32)
            vec.memset(state[:], 0)
            for sg in range(NSTRIP - 1, -1, -1):
                vec.tensor_copy(out=estrip[:, sg:sg + 1], in_=state[:])
                vec.tensor_scalar(out=ssh[:], in0=state[:], scalar1=3,
                                  scalar2=None, op0=ALU.mult)
                vec.tensor_tensor(out=sgg[:], in0=wst[:, sg:sg + 1],
                                  in1=ssh[:], op=ALU.logical_shift_right)
                vec.tensor_scalar(out=state[:], in0=sgg[:], scalar1=7,
                                  scalar2=None, op0=ALU.bitwise_and)
            nc.sync.dma_start(out=estrip_d, in_=estrip[:])
            tc.strict_bb_all_engine_barrier()
            eseed = pool.tile([128, 1], I32)
            nc.sync.dma_start(out=eseed[:],
                              in_=v(estrip_d, 0, [[1, NSTRIP], [NSTRIP, SEQ], [1, 1]]))

            # eg[g]: entry state into each group
            eg = pool.tile([128, NG], I32)
            st2 = pool.tile([128, 1], I32)
            esh = pool.tile([128, 1], I32)
            egg = pool.tile([128, 1], I32)
            vec.tensor_copy(out=st2[:], in_=eseed[:])
            for g in range(NG - 1, -1, -1):
                vec.tensor_copy(out=eg[:, g:g + 1], in_=st2[:])
                vec.tensor_scalar(out=esh[:], in0=st2[:], scalar1=3,
                                  scalar2=None, op0=ALU.mult)
                vec.tensor_tensor(out=egg[:], in0=Wm[:, g:g + 1],
                                  in1=esh[:], op=ALU.logical_shift_right)
                vec.tensor_scalar(out=st2[:], in0=egg[:], scalar1=7,
                                  scalar2=None, op0=ALU.bitwise_and)

            # ek[k]: entry state into each chunk
            ek = pool.tile([128, KL], I32)
            st3 = pool.tile([128, NG], I32)
            ksh = pool.tile([128, NG], I32)
            kgg = pool.tile([128, NG], I32)
            vec.tensor_copy(out=st3[:], in_=eg[:])
            for kk in range(G1 - 1, -1, -1):
                vec.tensor_copy(out=v(ek[:], kk, [P(ek), [G1, NG]]), in_=st3[:])
                vec.tensor_scalar(out=ksh[:], in0=st3[:], scalar1=3,
                                  scalar2=None, op0=ALU.mult)
                vec.tensor_tensor(
                    out=kgg[:],
                    in0=v(W1[:], kk, [P(W1), [G1, NG]]),
                    in1=ksh[:], op=ALU.logical_shift_right)
                vec.tensor_scalar(out=st3[:], in0=kgg[:], scalar1=7,
                                  scalar2=None, op0=ALU.bitwise_and)

            # final walk: re-derive per-t tags from B with known entries
            acc2 = pool.tile([128, STRIP], I32)
            stw = pool.tile([128, KL], I32)
            wsh = pool.tile([128, KL], I32)
            vec.tensor_copy(out=stw[:], in_=ek[:])
            for tl in range(S - 1, -1, -1):
                vec.tensor_scalar(out=wsh[:], in0=stw[:], scalar1=3,
                                  scalar2=None, op0=ALU.mult)
                vec.tensor_tensor(
                    out=stw[:],
                    in0=v(B_i[:], tl, [P(B_i), [S, KL]]),
                    in1=wsh[:], op=ALU.logical_shift_right)
                vec.tensor_scalar(out=stw[:], in0=stw[:], scalar1=7,
                                  scalar2=None, op0=ALU.bitwise_and)
                vec.tensor_copy(out=v(acc2[:], tl, [P(acc2), [S, KL]]),
                                in_=stw[:])

            # mask: path = (acc2 + 1) * (t < len) - 1
            maskf = pool.tile([128, STRIP], F32)
            vec.tensor_scalar(out=maskf[:], in0=tplane[:], scalar1=len_sb[:],
                              scalar2=None, op0=ALU.is_lt)
            acc2f = pool.tile([128, STRIP], F32)
            vec.tensor_copy(out=acc2f[:], in_=acc2[:])
            vec.tensor_scalar(out=acc2f[:], in0=acc2f[:], scalar1=1.0,
                              scalar2=None, op0=ALU.add)
            vec.tensor_tensor(out=acc2f[:], in0=acc2f[:], in1=maskf[:],
                              op=ALU.mult)
            vec.tensor_scalar(out=acc2f[:], in0=acc2f[:], scalar1=-1.0,
                              scalar2=None, op0=ALU.add)
            path_i = pool.tile([128, STRIP], I32)
            vec.tensor_copy(out=path_i[:], in_=acc2f[:])
            nc.sync.dma_start(out=path_d, in_=path_i[:])

        cpool2_cm.__exit__(None, None, None)
        vitpool_cm.__exit__(None, None, None)


def _build(L):
    import concourse.bacc as bacc
    import concourse.mybir as mybir
    from concourse import tile

    sh = _shapes(L)
    nc = bacc.Bacc("TRN2", target_bir_lowering=False, debug=False,
                   num_devices=NCORES)
    F32 = mybir.dt.float32
    I32 = mybir.dt.int32
    G = sh["G"]
    ins_aps = {
        "obs": nc.dram_tensor("obs", [SEQ, (L + 1) * C], F32, kind="ExternalInput").ap(),
        "obs_lane": nc.dram_tensor("obs_lane", [128, G * (SC + HALO) * C], F32,
                                   kind="ExternalInput").ap(),
        "trep_h": nc.dram_tensor("trep_h", [128, G * C * C], F32,
                                 kind="ExternalInput").ap(),
        "trep": nc.dram_tensor("trep", [128, C * C], F32, kind="ExternalInput").ap(),
        "wdesc": nc.dram_tensor("wdesc", [128, C], F32, kind="ExternalInput").ap(),
        "tplane": nc.dram_tensor("tplane", [128, sh["STRIP"]], F32, kind="ExternalInput").ap(),
        "len_col": nc.dram_tensor("len_col", [128, 1], F32, kind="ExternalInput").ap(),
        "lenm1": nc.dram_tensor("lenm1", [128, 1], F32, kind="ExternalInput").ap(),
        "pow8f": nc.dram_tensor("pow8f", [128, C], F32, kind="ExternalInput").ap(),
        "pow8nf": nc.dram_tensor("pow8nf", [128, C], F32, kind="ExternalInput").ap(),
        "pow8i": nc.dram_tensor("pow8i", [128, C], I32, kind="ExternalInput").ap(),
        "eci": nc.dram_tensor("eci", [128, C * sh["KL"]], I32, kind="ExternalInput").ap(),
        "obs_lane16": nc.dram_tensor("obs_lane16", [128, sh["G"] * (SC + HALO) * C],
                                     mybir.dt.float16, kind="ExternalInput").ap(),
        "trep16": nc.dram_tensor("trep16", [128, C * C], mybir.dt.float16,
                                 kind="ExternalInput").ap(),
        "trep_h16": nc.dram_tensor("trep_h16", [128, sh["G"] * C * C],
                                   mybir.dt.float16, kind="ExternalInput").ap(),
    }
    outs_aps = {"path": nc.dram_tensor("path", [128, sh["STRIP"]], mybir.dt.int32,
                                       kind="ExternalOutput").ap()}
    with tile.TileContext(nc) as tc:
        _emit(tc, ins_aps, outs_aps, L)
    nc.compile()
    return nc


def kernel(observes, transitions, lengths):
    from concourse.bass_utils import run_bass_kernel_spmd

    observes = np.asarray(observes, np.float32)
    transitions = np.asarray(transitions, np.float32)
    lengths_np = np.asarray(lengths)
    L = observes.shape[2]

    if L not in _CACHE:
        _CACHE[L] = _build(L)
    nc = _CACHE[L]

    in_maps = [
        _host_prep(observes[SEQ * c:SEQ * (c + 1)], transitions,
                   lengths_np[SEQ * c:SEQ * (c + 1)], L)
        for c in range(NCORES)
    ]
    res = run_bass_kernel_spmd(nc, in_maps, core_ids=list(range(NCORES)))
    out = np.concatenate(
        [_host_post(res.results[c]["path"], L) for c in range(NCORES)], 0)
    return out.astype(np.int32)
